# revision 1
# baseline (speedup 1.0000x reference)
"""Trainium2 Bass kernel for a dense transformer block (B=4, T=2048, C=1024, 16 heads).

Sharding over 8 NeuronCores: core i handles batch b=i//2 with shard s=i%2.
 - LN1 + QKV + causal attention for its 8 heads (c-slice [512s, 512s+512)) over full T
 - exchange of attention outputs within the (b) pair via 4 chunked
   ReduceScatter ops (zero-padded concat trick, fully SPMD-symmetric)
 - proj + LN2 + FFN + residuals on its t-half rows [1024s, 1024s+1024)

All GEMMs run in float32r (fp22 multiply, fp32 accumulate). LayerNorm
gain/bias are folded into the weight matrices on the host.
"""

from contextlib import ExitStack

import ml_dtypes
import numpy as np

import concourse.bass as bass
import concourse.mybir as mybir
import concourse.tile as tile
from concourse import bacc
from concourse.bass_utils import run_bass_kernel_spmd

f32 = mybir.dt.float32
f32r = mybir.dt.float32r
bf16 = mybir.dt.bfloat16
AF = mybir.ActivationFunctionType
ALU = mybir.AluOpType
AX = mybir.AxisListType

B, T, C = 4, 2048, 1024
NH, D = 16, 64
F = 4 * C
H = C // 2            # per-core head c-slice (8 heads)
TH = T // 2           # per-core t-half for proj/FFN
EPS = 1e-5
RG = [[0, 1], [2, 3], [4, 5], [6, 7]]

_CACHE = {}


class S:
    """Shared build state."""
    pass


def _layernorm_tile(nc, st, xt, dst, sq_pool, sq_tag):
    """Row-standardize xt [128, C] -> dst [128, C] f32r (dst doubles as scratch)."""
    work = st.work
    s1 = work.tile([128, 1], f32, name="s1", tag="s1")
    s2 = work.tile([128, 1], f32, name="s2", tag="s2")
    sq = sq_pool.tile([128, C], f32, name="sq", tag=sq_tag)
    nc.vector.reduce_sum(s1[:], xt[:], axis=AX.X)
    nc.scalar.activation(sq[:], xt[:], AF.Square, accum_out=s2[:])
    mu = work.tile([128, 1], f32, name="mu", tag="mu")
    var = work.tile([128, 1], f32, name="var", tag="var")
    nc.vector.tensor_scalar_mul(mu[:], s1[:], 1.0 / C)
    nc.vector.tensor_scalar_mul(s2[:], s2[:], 1.0 / C)
    nc.vector.tensor_tensor(var[:], mu[:], mu[:], ALU.mult)
    nc.vector.tensor_tensor(var[:], s2[:], var[:], ALU.subtract)
    nc.vector.tensor_scalar_add(var[:], var[:], EPS)
    sd = work.tile([128, 1], f32, name="sd", tag="sd")
    nc.scalar.activation(sd[:], var[:], AF.Sqrt)
    rsig = work.tile([128, 1], f32, name="rsig", tag="rsig")
    with nc.allow_low_precision(reason="LN rsqrt"):
        nc.vector.reciprocal(rsig[:], sd[:])
    nmu = work.tile([128, 1], f32, name="nmu", tag="nmu")
    nc.vector.tensor_tensor(nmu[:], mu[:], rsig[:], ALU.mult)
    nc.vector.tensor_scalar_mul(nmu[:], nmu[:], -1.0)
    nc.scalar.activation(dst[:], xt[:], AF.Identity, bias=nmu[:], scale=rsig[:])


def _phase_qkv(nc, st):
    """LN1, transpose, QKV GEMMs. Fills st.qT, st.kT, st.vn."""
    ps_t, work = st.ps_t, st.work
    st.qkvp = st.tc.tile_pool(name="qkv", bufs=1)
    qkv = st.qkvp.__enter__()
    st.wqkvp = st.tc.tile_pool(name="wqkv", bufs=1)
    wqkv = st.wqkvp.__enter__()
    st.xhp = st.tc.tile_pool(name="xh", bufs=2)
    xh = st.xhp.__enter__()
    st.htcp = st.tc.tile_pool(name="htc", bufs=1)
    htc = st.htcp.__enter__()

    wq_sb = [wqkv.tile([128, H], bf16, name=f"wq{k}", tag=f"wq{k}") for k in range(8)]
    wk_sb = [wqkv.tile([128, H], bf16, name=f"wk{k}", tag=f"wk{k}") for k in range(8)]
    wv_sb = [wqkv.tile([128, H], bf16, name=f"wv{k}", tag=f"wv{k}") for k in range(8)]
    for k in range(8):
        nc.sync.dma_start(wq_sb[k][:], st.wq_h[k * 128:(k + 1) * 128, :])
        nc.sync.dma_start(wk_sb[k][:], st.wk_h[k * 128:(k + 1) * 128, :])
        nc.sync.dma_start(wv_sb[k][:], st.wv_h[k * 128:(k + 1) * 128, :])

    st.qT = [qkv.tile([128, T], bf16, name=f"qT{i}", tag=f"qT{i}") for i in range(4)]
    st.kT = [qkv.tile([128, T], bf16, name=f"kT{i}", tag=f"kT{i}") for i in range(4)]
    st.vn = [qkv.tile([128, 520], bf16, name=f"vn{i}", tag=f"vn{i}")
             for i in range(16)]

    for j in range(4):  # t-chunks of 512
        hcol = htc.tile([128, 8 * 512], bf16, name="hcol", tag="hcol")
        for tt4 in range(4):  # t-tiles of 128 within the chunk
            tt = j * 4 + tt4
            xt = xh.tile([128, C], f32, name="xt", tag="xt")
            nc.sync.dma_start(xt[:], st.x_h[tt * 128:(tt + 1) * 128, :])
            ht = xh.tile([128, C], bf16, name="ht", tag="ht")
            _layernorm_tile(nc, st, xt, ht, xh, "sq")
            for cc in range(8):
                ptr = ps_t("tr", (128, 128), bf16)
                nc.tensor.transpose(ptr[:], ht[:, cc * 128:(cc + 1) * 128],
                                    st.ident[:])
                nc.vector.tensor_copy(
                    out=hcol[:, cc * 512 + tt4 * 128:cc * 512 + (tt4 + 1) * 128],
                    in_=ptr[:])
        # q/k GEMMs for this t-chunk
        for dst, wsb, bsb in ((st.qT, wq_sb, st.bq_sb), (st.kT, wk_sb, st.bk_sb)):
            for co in range(4):
                pg = ps_t("gemm")
                for k in range(8):
                    nc.tensor.matmul(pg[:], wsb[k][:, co * 128:(co + 1) * 128],
                                     hcol[:, k * 512:(k + 1) * 512],
                                     start=(k == 0), stop=(k == 7))
                nc.scalar.activation(dst[co][:, j * 512:(j + 1) * 512], pg[:],
                                     AF.Identity, bias=bsb[:, co:co + 1])
        # v GEMM (natural layout, strided into vn with ones columns)
        for tt4 in range(4):
            tt = j * 4 + tt4
            pg = ps_t("gemm")
            for k in range(8):
                nc.tensor.matmul(
                    pg[:], hcol[:, k * 512 + tt4 * 128:k * 512 + (tt4 + 1) * 128],
                    wv_sb[k][:], start=(k == 0), stop=False)
            nc.tensor.matmul(pg[:], st.onesr[:, 0:128], st.bv_sb[:],
                             start=False, stop=True)
            nc.scalar.copy(
                st.vn[tt][:, 0:520].rearrange("p (h e) -> p h e", h=8)[:, :, 0:64],
                pg[:].rearrange("p (h d) -> p h d", h=8))
            nc.sync.dma_start(
                st.vn[tt][:, 0:520].rearrange("p (h e) -> p h e", h=8)[:, :, 64:65],
                st.ones8[:].rearrange("p (h o) -> p h o", h=8))


def _phase_attention(nc, st):
    """Causal attention for 8 local heads; ships results via ReduceScatter."""
    st.htcp.__exit__(None, None, None)
    st.xhp.__exit__(None, None, None)
    st.wqkvp.__exit__(None, None, None)
    ps_t, work = st.ps_t, st.work
    st.wop = st.tc.tile_pool(name="wop", bufs=1, side="right")
    wop = st.wop.__enter__()
    st.attp = st.tc.tile_pool(name="attp", bufs=1, side="right")
    attp = st.attp.__enter__()

    attA = [attp.tile([128, T], bf16, name=f"attA{i}", tag=f"attA{i}")
            for i in range(4)]
    st.attA = attA
    aw = st.tc.tile_pool(name="aw", bufs=2)
    st.awp = aw
    aw = aw.__enter__()
    st.wo_sb = [wop.tile([128, C], bf16, name=f"wo{k}", tag=f"wo{k}")
                for k in range(8)]
    for k in range(8):
        nc.sync.dma_start(st.wo_sb[k][:], st.wo_h[k * 128:(k + 1) * 128, :])

    if _CACHE.get("debug"):
        nc.sync.dma_start(st.dq_h[:], st.qT[0][:].bitcast(f32))
    for hp in range(4):
        for j in range(4):
            tq0 = j * 512
            nk = 4 * (j + 1)
            po = [ps_t("pvA"), ps_t("pvB")]
            for kk in range(nk):
                r = 128 * (kk - 4 * j) if kk >= 4 * j else 0
                pqk = ps_t("qkp", (128, 1024))
                for bi, b0 in enumerate((0, 64)):
                    nc.tensor.matmul(
                        pqk[:, bi * 512 + r:bi * 512 + 512],
                        st.kT[hp][b0:b0 + 64, kk * 128:(kk + 1) * 128],
                        st.qT[hp][b0:b0 + 64, tq0 + r:tq0 + 512],
                        start=True, stop=True)
                ptb = st.ptp.tile([128, 1024], bf16, name="ptb", tag="pt")
                if r == 0:
                    nc.scalar.activation(ptb[:], pqk[:], AF.Exp)
                else:
                    nc.scalar.activation(
                        ptb[:].rearrange("p (b w) -> p b w", b=2)[:, :, r:512],
                        pqk[:].rearrange("p (b w) -> p b w", b=2)[:, :, r:512],
                        AF.Exp)
                if kk >= 4 * j:
                    nc.vector.tensor_tensor(
                        ptb[:].rearrange("p (b w) -> p b w", b=2)[:, :, r:r + 128],
                        ptb[:].rearrange("p (b w) -> p b w", b=2)[:, :, r:r + 128],
                        st.tri[:, None, :].to_broadcast((128, 2, 128)),
                        ALU.mult)
                for bi in range(2):
                    h = 2 * hp + bi
                    nc.tensor.matmul(
                        po[bi][0:65, r:512],
                        st.vn[kk][:, 65 * h:65 * h + 65],
                        ptb[:, bi * 512 + r:bi * 512 + 512],
                        start=(kk == 0), stop=(kk == nk - 1))
            sj = j // 2
            for bi, b0 in enumerate((0, 64)):
                rs_row = aw.tile([1, 512], bf16, name="rs_row", tag="rsrow")
                nc.scalar.copy(rs_row[:], po[bi][64:65, :])
                pb = ps_t("gemm", (64, 512))
                nc.tensor.matmul(pb[:], st.onesr[:, 0:64], rs_row[:],
                                 start=True, stop=True)
                rbi = aw.tile([64, 512], f32, name="rbi", tag="rbi")
                nc.vector.reciprocal_approx_fast(rbi[:], pb[:])
                rbiA = aw.tile([64, 512], f32, name="rbiA", tag="rbiA")
                rbiB = aw.tile([64, 512], f32, name="rbiB", tag="rbiB")
                nc.vector.tensor_scalar_mul(rbiA[:], rbi[:],
                                            st.sel_sb[0:64, sj:sj + 1])
                nc.vector.tensor_scalar_mul(rbiB[:], rbi[:],
                                            st.seln_sb[0:64, sj:sj + 1])
                nc.vector.tensor_tensor(
                    attA[hp][b0:b0 + 64, tq0:tq0 + 512],
                    po[bi][0:64, :], rbiA[:], ALU.mult)
                attBc = aw.tile([64, 512], bf16, name="attBc", tag="attBc")
                nc.vector.tensor_tensor(attBc[:], po[bi][0:64, :], rbiB[:],
                                        ALU.mult)
                nc.sync.dma_start(
                    st.rs_in[hp][sj, 128 + b0:128 + b0 + 64,
                                 (j % 2) * 512:(j % 2) * 512 + 512],
                    attBc[:])

        for s in range(2):
            nc.sync.dma_start(st.rs_in[hp][s, 0:128, :],
                              attA[hp][:, s * TH:(s + 1) * TH])
        nc.gpsimd.collective_compute(
            "ReduceScatter", ALU.add, replica_groups=RG,
            ins=[st.rs_in[hp][:]], outs=[st.rs_out[hp][:]])


def _phase_proj(nc, st):
    """Gather exchanged attention, projection, residual. Fills st.x2."""
    ps_t = st.ps_t
    st.awp.__exit__(None, None, None)
    st.qkvp.__exit__(None, None, None)
    st.x2p = st.tc.tile_pool(name="x2p", bufs=1)
    x2p = st.x2p.__enter__()
    st.latebp = st.tc.tile_pool(name="lateb", bufs=1)
    lateb = st.latebp.__enter__()
    st.attsbp = st.tc.tile_pool(name="attsb", bufs=1)
    attsb = st.attsbp.__enter__()
    st.xrpp = st.tc.tile_pool(name="xrp", bufs=2)
    xrp = st.xrpp.__enter__()

    st.b2_sb = lateb.tile([1, C], bf16, name="b2_sb")
    nc.sync.dma_start(st.b2_sb[:], st.b2_h[:])
    att_sb = [attsb.tile([128, TH], bf16, name=f"asb{k}", tag=f"asb{k}")
              for k in range(4)]
    for hp in range(4):
        nc.sync.dma_start(att_sb[hp][:], st.rs_out[hp][128:256, :])
    st.x2 = [x2p.tile([128, C], f32, name=f"x2_{t}", tag=f"x2_{t}")
             for t in range(8)]
    for tt in range(8):
        xr = xrp.tile([128, C], f32, name="xr", tag="xr")
        nc.sync.dma_start(xr[:], st.xres_h[tt * 128:(tt + 1) * 128, :])
        for cc in range(2):
            pg = ps_t("gemm")
            for k in range(4):
                for half in range(2):
                    nc.tensor.matmul(
                        pg[:],
                        st.attA[k][:, half * TH + tt * 128:
                                   half * TH + (tt + 1) * 128],
                        st.wo_sb[k][:, cc * 512:(cc + 1) * 512],
                        start=(k == 0 and half == 0), stop=False)
            for k in range(4):
                nc.tensor.matmul(pg[:], att_sb[k][:, tt * 128:(tt + 1) * 128],
                                 st.wo_sb[4 + k][:, cc * 512:(cc + 1) * 512],
                                 start=False, stop=(k == 3))
            nc.vector.tensor_tensor(st.x2[tt][:, cc * 512:(cc + 1) * 512],
                                    pg[:], xr[:, cc * 512:(cc + 1) * 512],
                                    ALU.add)
    # free proj-side pools (xrp/attsb LIFO on left; wop on right stack)
    st.xrpp.__exit__(None, None, None)
    st.attsbp.__exit__(None, None, None)
    st.attp.__exit__(None, None, None)
    st.wop.__exit__(None, None, None)


def _phase_ln2(nc, st):
    """LN2 + transpose to h2T."""
    ps_t = st.ps_t
    st.h2p = st.tc.tile_pool(name="h2p", bufs=1)
    h2p = st.h2p.__enter__()
    st.h2wp = st.tc.tile_pool(name="h2w", bufs=2)
    h2w = st.h2wp.__enter__()
    if _CACHE.get("debug"):
        for tt in range(8):
            nc.sync.dma_start(st.dx2_h[tt * 128:(tt + 1) * 128, :], st.x2[tt][:])
    st.h2T = [h2p.tile([128, TH], bf16, name=f"h2T{k}", tag=f"h2T{k}")
              for k in range(8)]
    for tt in range(8):
        h2t = h2w.tile([128, C], bf16, name="h2t", tag="h2t")
        _layernorm_tile(nc, st, st.x2[tt], h2t, h2w, "sqb")
        for cc in range(8):
            ptr = ps_t("tr", (128, 128), bf16)
            nc.tensor.transpose(ptr[:], h2t[:, cc * 128:(cc + 1) * 128],
                                st.ident[:])
            nc.vector.tensor_copy(out=st.h2T[cc][:, tt * 128:(tt + 1) * 128],
                                  in_=ptr[:])


def _phase_ffn(nc, st):
    """FFN with grouped ff-dim accumulation, residual, output DMA."""
    st.h2wp.__exit__(None, None, None)
    ps_t = st.ps_t
    yacp = st.tc.tile_pool(name="yac", bufs=1)
    yac = yacp.__enter__()
    w1pp = st.tc.tile_pool(name="w1p", bufs=3)
    w1p = w1pp.__enter__()
    w2pp = st.tc.tile_pool(name="w2p", bufs=8)
    w2p = w2pp.__enter__()
    utpp = st.tc.tile_pool(name="utp", bufs=8)
    utp = utpp.__enter__()

    y_acc = [yac.tile([128, C], f32, name=f"ya{t}", tag=f"ya{t}")
             for t in range(8)]
    for g in range(4):
        ut_g = []
        for ff in range(8):
            f = g * 8 + ff
            w1c = w1p.tile([128, 8, 128], bf16, name="w1c", tag="w1c")
            for k in range(8):
                nc.sync.dma_start(w1c[:, k, :],
                                  st.w1_h[k * 128:(k + 1) * 128,
                                          f * 128:(f + 1) * 128])
            ut = utp.tile([128, TH], bf16, name="ut", tag="ut")
            for tch in range(2):
                pg = ps_t("gemm")
                for k in range(8):
                    nc.tensor.matmul(pg[:], w1c[:, k, :],
                                     st.h2T[k][:, tch * 512:(tch + 1) * 512],
                                     start=(k == 0), stop=(k == 7))
                nc.scalar.activation(ut[:, tch * 512:(tch + 1) * 512], pg[:],
                                     AF.Relu, bias=st.b1_sb[:, f:f + 1])
            ut_g.append(ut)
        w2g = []
        for ff in range(8):
            f = g * 8 + ff
            w2t = w2p.tile([128, C], bf16, name="w2t", tag="w2t")
            nc.sync.dma_start(w2t[:], st.w2_h[f * 128:(f + 1) * 128, :])
            w2g.append(w2t)
        for tt in range(8):
            for cc in range(2):
                pg = ps_t("gemm")
                for ff in range(8):
                    nc.tensor.matmul(pg[:], ut_g[ff][:, tt * 128:(tt + 1) * 128],
                                     w2g[ff][:, cc * 512:(cc + 1) * 512],
                                     start=(ff == 0),
                                     stop=(False if g == 0 else ff == 7))
                if g == 0:
                    nc.tensor.matmul(pg[:], st.onesr[:, 0:128],
                                     st.b2_sb[:, cc * 512:(cc + 1) * 512],
                                     start=False, stop=True)
                    nc.vector.tensor_tensor(
                        y_acc[tt][:, cc * 512:(cc + 1) * 512], pg[:],
                        st.x2[tt][:, cc * 512:(cc + 1) * 512], ALU.add)
                else:
                    nc.vector.tensor_tensor(
                        y_acc[tt][:, cc * 512:(cc + 1) * 512], pg[:],
                        y_acc[tt][:, cc * 512:(cc + 1) * 512], ALU.add)
    for tt in range(8):
        nc.sync.dma_start(st.y_h[tt * 128:(tt + 1) * 128, :], y_acc[tt][:])
    utpp.__exit__(None, None, None)
    w2pp.__exit__(None, None, None)
    w1pp.__exit__(None, None, None)
    yacp.__exit__(None, None, None)
    st.h2p.__exit__(None, None, None)
    st.latebp.__exit__(None, None, None)
    st.x2p.__exit__(None, None, None)


def build_program():
    if "nc" in _CACHE:
        return _CACHE["nc"]
    nc = bacc.Bacc(None)
    st = S()

    st.x_h = nc.declare_dram_parameter("x", [T, C], f32, isOutput=False)
    st.xres_h = nc.declare_dram_parameter("xres", [TH, C], f32, isOutput=False)
    st.wq_h = nc.declare_dram_parameter("wq", [C, H], bf16, isOutput=False)
    st.wk_h = nc.declare_dram_parameter("wk", [C, H], bf16, isOutput=False)
    st.wv_h = nc.declare_dram_parameter("wv", [C, H], bf16, isOutput=False)
    bq_h = nc.declare_dram_parameter("bq", [128, 4], f32, isOutput=False)
    bk_h = nc.declare_dram_parameter("bk", [128, 4], f32, isOutput=False)
    bv_h = nc.declare_dram_parameter("bv", [1, H], bf16, isOutput=False)
    st.wo_h = nc.declare_dram_parameter("wo", [C, C], bf16, isOutput=False)
    st.w1_h = nc.declare_dram_parameter("w1", [C, F], bf16, isOutput=False)
    b1_h = nc.declare_dram_parameter("b1", [128, 32], f32, isOutput=False)
    st.w2_h = nc.declare_dram_parameter("w2", [F, C], bf16, isOutput=False)
    b2_h = nc.declare_dram_parameter("b2", [1, C], bf16, isOutput=False)
    ident_h = nc.declare_dram_parameter("ident", [128, 128], bf16, isOutput=False)
    tri_h = nc.declare_dram_parameter("tri", [128, 128], bf16, isOutput=False)
    onesr_h = nc.declare_dram_parameter("onesr", [1, 128], bf16, isOutput=False)
    ones8_h = nc.declare_dram_parameter("ones8", [128, 8], bf16, isOutput=False)
    sel_h = nc.declare_dram_parameter("sel", [128, 2], f32, isOutput=False)
    seln_h = nc.declare_dram_parameter("seln", [128, 2], f32, isOutput=False)
    st.y_h = nc.declare_dram_parameter("y", [TH, C], f32, isOutput=True)
    if _CACHE.get("debug"):
        st.dq_h = nc.declare_dram_parameter("dbg_q", [128, T], f32, isOutput=True)
        st.da_h = nc.declare_dram_parameter("dbg_att", [128, T], f32, isOutput=True)
        st.dx2_h = nc.declare_dram_parameter("dbg_x2", [TH, C], f32, isOutput=True)

    st.rs_in = [nc.dram_tensor(f"rs_in{hp}", [2, 256, TH], bf16)
                for hp in range(4)]
    st.rs_out = [nc.dram_tensor(f"rs_out{hp}", [256, TH], bf16)
                 for hp in range(4)]

    with tile.TileContext(nc) as tc, ExitStack() as stack:
        st.tc, st.stack = tc, stack
        cst = stack.enter_context(tc.tile_pool(name="const", bufs=1))
        ps = stack.enter_context(tc.tile_pool(name="ps", bufs=1, space="PSUM"))
        st.work = stack.enter_context(tc.tile_pool(name="work", bufs=2))
        st.ptp = stack.enter_context(tc.tile_pool(name="ptp", bufs=2))

        st.ident = cst.tile([128, 128], bf16, name="ident")
        st.tri = cst.tile([128, 128], bf16, name="tri")
        st.onesr = cst.tile([1, 128], bf16, name="onesr")
        st.ones8 = cst.tile([128, 8], bf16, name="ones8")
        st.bq_sb = cst.tile([128, 4], f32, name="bq_sb")
        st.bk_sb = cst.tile([128, 4], f32, name="bk_sb")
        st.bv_sb = cst.tile([1, H], bf16, name="bv_sb")
        st.b1_sb = cst.tile([128, 32], f32, name="b1_sb")
        st.sel_sb = cst.tile([128, 2], f32, name="sel_sb")
        st.seln_sb = cst.tile([128, 2], f32, name="seln_sb")
        for t_, h_ in [(st.ident, ident_h), (st.tri, tri_h), (st.onesr, onesr_h),
                       (st.ones8, ones8_h), (st.bq_sb, bq_h), (st.bk_sb, bk_h),
                       (st.bv_sb, bv_h), (st.b1_sb, b1_h),
                       (st.sel_sb, sel_h), (st.seln_sb, seln_h)]:
            nc.sync.dma_start(t_[:], h_[:])
        st.b2_h = b2_h

        def ps_t(tag, shape=(128, 512), dt=f32):
            return ps.tile(list(shape), dt, tag=tag, name=f"ps_{tag}")
        st.ps_t = ps_t

        _phase_qkv(nc, st)
        _phase_attention(nc, st)
        _phase_proj(nc, st)
        _phase_ln2(nc, st)
        _phase_ffn(nc, st)

    nc.compile()
    _CACHE["nc"] = nc
    return nc


def make_inputs(x, Wq, Wk, Wv, Wo, bo, W1, b1, W2, b2,
                ln1_g, ln1_b, ln2_g, ln2_b):
    """Build per-core input maps (host-side sharding + LN folding)."""
    x = np.asarray(x, np.float32)
    scale = float(C) ** -0.5

    wq_eff = ln1_g[:, None] * Wq
    wk_eff = ln1_g[:, None] * Wk * scale
    wv_eff = ln1_g[:, None] * Wv
    bq_full = ln1_b @ Wq
    bk_full = (ln1_b @ Wk) * scale
    bv_full = ln1_b @ Wv
    w1_eff = ln2_g[:, None] * W1
    b1_eff = b1 + ln2_b @ W1

    BF = ml_dtypes.bfloat16
    ident = np.eye(128, dtype=BF)
    tri = np.triu(np.ones((128, 128), BF))
    onesr = np.ones((1, 128), BF)
    ones8 = np.ones((128, 8), BF)

    in_maps = []
    for core in range(8):
        b, s = core // 2, core % 2
        cs = slice(s * H, (s + 1) * H)
        ts = slice(s * TH, (s + 1) * TH)
        own = np.arange(s * H, (s + 1) * H)
        other = np.arange((1 - s) * H, (2 - s) * H)
        perm = np.concatenate([own, other])
        in_maps.append({
            "x": np.ascontiguousarray(x[b]),
            "xres": np.ascontiguousarray(x[b, ts, :] + bo[None, :]),
            "wq": np.ascontiguousarray(wq_eff[:, cs].astype(BF)),
            "wk": np.ascontiguousarray(wk_eff[:, cs].astype(BF)),
            "wv": np.ascontiguousarray(wv_eff[:, cs].astype(BF)),
            "bq": np.ascontiguousarray(bq_full[cs].reshape(4, 128).T),
            "bk": np.ascontiguousarray(bk_full[cs].reshape(4, 128).T),
            "bv": np.ascontiguousarray(bv_full[cs].reshape(1, H).astype(BF)),
            "wo": np.ascontiguousarray(Wo[perm, :].astype(BF)),
            "w1": np.ascontiguousarray(w1_eff.astype(BF)),
            "b1": np.ascontiguousarray(b1_eff.reshape(32, 128).T),
            "w2": np.ascontiguousarray(W2.astype(BF)),
            "b2": np.ascontiguousarray(b2.reshape(1, C).astype(BF)),
            "ident": ident, "tri": tri, "onesr": onesr, "ones8": ones8,
            "sel": np.tile(np.eye(2, dtype=np.float32)[s][None, :], (128, 1)),
            "seln": np.tile(np.eye(2, dtype=np.float32)[1 - s][None, :], (128, 1)),
        })
    return in_maps


def kernel(**inputs):
    nc = build_program()
    in_maps = make_inputs(**{k: np.asarray(v, np.float32) for k, v in inputs.items()})
    res = run_bass_kernel_spmd(nc, in_maps, list(range(8)))
    out = np.empty((B, T, C), np.float32)
    for core in range(8):
        b, s = core // 2, core % 2
        out[b, s * TH:(s + 1) * TH, :] = res.results[core]["y"]
    return out



# revision 27
# speedup vs baseline: 1.0497x; 1.0497x over previous
"""Trainium2 Bass kernel for a dense transformer block (B=4, T=2048, C=1024, 16 heads).

Sharding over 8 NeuronCores: core i handles batch b=i//2 with shard s=i%2.
 - LN1 + QKV + causal attention for its 8 heads (c-slice [512s, 512s+512)) over full T
 - exchange of attention outputs within the (b) pair via 4 per-head-pair
   ReduceScatter ops (zero-masked, SPMD-symmetric)
 - proj + LN2 + FFN + residuals on its t-half rows [1024s, 1024s+1024)

All activations flow in TRANSPOSED layout [C, T] (host passes xT / xresT and
un-transposes y), which removes every PE transpose.  LayerNorm is computed as
column statistics (partition-sum matmuls) + broadcast normalize on DVE.
LayerNorm gains/biases are folded into the weights on the host.
"""

from contextlib import ExitStack

import ml_dtypes
import numpy as np

import concourse.bass as bass
import concourse.mybir as mybir
import concourse.tile as tile
from concourse import bacc
from concourse.bass_utils import run_bass_kernel_spmd

f32 = mybir.dt.float32
bf16 = mybir.dt.bfloat16
AF = mybir.ActivationFunctionType
ALU = mybir.AluOpType
AX = mybir.AxisListType

B, T, C = 4, 2048, 1024
NH, D = 16, 64
F = 4 * C
H = C // 2            # per-core head c-slice (8 heads)
TH = T // 2           # per-core t-half for proj/FFN
NC_CHUNK = 512        # t-chunk for qkv/attention
EPS = 1e-5
RG = [[0, 1], [2, 3], [4, 5], [6, 7]]

_CACHE = {}


class S:
    pass


def _stats_and_norm(nc, st, src_tiles, n_src, dst_tiles, sq_tag, pool, sq_dt,
                    ones_lhs):
    """Column LayerNorm: src [128, 512] tiles covering 1024 c-rows -> dst bf16
    tiles (x - mu) * rsig per column.  sq_dt/ones_lhs match the src dtype so
    matmul operand dtypes agree."""
    work = st.work
    # squares on DVE
    sqs = []
    for k in range(n_src):
        sq = pool.tile([128, 512], sq_dt, name="sq", tag=f"{sq_tag}{k % 2}")
        nc.vector.tensor_tensor(sq[:], src_tiles[k][:], src_tiles[k][:], ALU.mult)
        sqs.append(sq)
    # partition sums via M=1 matmuls; separate psum groups for sum and sumsq
    # (sharing a bank risks a fatal PE-write/DVE-read same-bank collision)
    pst = st.ps_aux((1, 512))
    psq = st.ps_aux((1, 512))
    for k in range(n_src):
        nc.tensor.matmul(pst[:], ones_lhs[:], src_tiles[k][:],
                         start=(k == 0), stop=(k == n_src - 1))
        nc.tensor.matmul(psq[:], ones_lhs[:], sqs[k][:],
                         start=(k == 0), stop=(k == n_src - 1))
    # row math (mu/sd rows in bf16 so the broadcast matmuls match onesr's dtype)
    mu = st.rows.tile([1, 512], bf16, name="mu_r", tag="mu_r")
    msq = st.rows.tile([1, 512], f32, name="msq_r", tag="msq_r")
    var = st.rows.tile([1, 512], f32, name="var_r", tag="var_r")
    sd = st.rows.tile([1, 512], bf16, name="sd_r", tag="sd_r")
    nc.vector.tensor_scalar_mul(mu[:], pst[:], 1.0 / C)
    nc.vector.tensor_scalar_mul(msq[:], psq[:], 1.0 / C)
    nc.vector.tensor_tensor(var[:], mu[:], mu[:], ALU.mult)
    nc.vector.tensor_tensor(var[:], msq[:], var[:], ALU.subtract)
    nc.vector.tensor_scalar_add(var[:], var[:], EPS)
    nc.scalar.activation(sd[:], var[:], AF.Sqrt)
    # broadcasts
    pmu = st.ps_aux((128, 512))
    nc.tensor.matmul(pmu[:], st.onesr[:, 0:128], mu[:], start=True, stop=True)
    mu_b = work.tile([128, 512], bf16, name="mu_b", tag="mu_b")
    nc.vector.tensor_copy(out=mu_b[:], in_=pmu[:])
    psd = st.ps_aux((128, 512))
    nc.tensor.matmul(psd[:], st.onesr[:, 0:128], sd[:], start=True, stop=True)
    rsig_b = work.tile([128, 512], f32, name="rsig_b", tag="rsig_b")
    nc.vector.reciprocal_approx_fast(rsig_b[:], psd[:])
    # normalize
    for k in range(n_src):
        dif = pool.tile([128, 512], bf16, name="dif", tag=f"{sq_tag}{k % 4}")
        nc.vector.tensor_tensor(dif[:], src_tiles[k][:], mu_b[:], ALU.subtract)
        nc.vector.tensor_tensor(dst_tiles[k][:], dif[:], rsig_b[:], ALU.mult)


def _emit_qkv_chunk(nc, st, jc):
    """LN1 stats + q/k/v GEMMs for t-chunk jc (columns [jc*512, jc*512+512))."""
    xh, ps_g = st.xh, st.ps_g
    xt = []
    for k in range(8):
        t_ = xh.tile([128, 512], bf16, name="xt", tag=f"xt{k}")
        nc.sync.dma_start(t_[:], st.xt_h[k * 128:(k + 1) * 128,
                                          jc * 512:(jc + 1) * 512])
        xt.append(t_)
    hT = [xh.tile([128, 512], bf16, name="hT", tag=f"hT{k}") for k in range(8)]
    _stats_and_norm(nc, st, xt, 8, hT, "sq", xh, bf16, st.onescol)

    # q, k GEMMs -> transposed layout [c' 128, t 512]
    for dst, wsb, bsb in ((st.qT, st.wq_sb, st.bq_sb), (st.kT, st.wk_sb, st.bk_sb)):
        for co in range(4):
            pg = ps_g()
            for k in range(8):
                nc.tensor.matmul(pg[:], wsb[k][:, co * 128:(co + 1) * 128],
                                 hT[k][:], start=(k == 0), stop=(k == 7))
            nc.vector.tensor_scalar_add(dst[co][:, jc * 512:(jc + 1) * 512],
                                        pg[:], bsb[:, co:co + 1])
    # v GEMM -> natural layout [t 128, cv 512], strided into vn (65-col heads)
    for tt4 in range(4):
        tt = jc * 4 + tt4
        pg = ps_g()
        for k in range(8):
            nc.tensor.matmul(pg[:], hT[k][:, tt4 * 128:(tt4 + 1) * 128],
                             st.wv_sb[k][:], start=(k == 0), stop=False)
        nc.tensor.matmul(pg[:], st.onesr[:, 0:128], st.bv_sb[:],
                         start=False, stop=True)
        nc.vector.tensor_copy(
            out=st.vn[tt][:, 0:520].rearrange("p (h e) -> p h e", h=8)[:, :, 0:64],
            in_=pg[:].rearrange("p (h d) -> p h d", h=8))


def _emit_attn_chunk(nc, st, j):
    """Causal attention of q-chunk j against k-chunks 0..j, all 4 head-pairs.
    Writes the own-t-half part into compact attA and DMAs the peer part into
    rs_in; fires the per-hp ReduceScatter after the last chunk."""
    aw = st.aw
    tq0 = j * 512
    nk = 4 * (j + 1)
    sj = j // 2        # which t-half this q-chunk belongs to
    lc = (j % 2) * 512  # column inside the compact [128, TH] buffers
    for hp in range(4):
        po = [st.ps_po(0), st.ps_po(1)]
        for kk in range(nk):
            r = 128 * (kk - 4 * j) if kk >= 4 * j else 0
            pqk = st.ps_qk()
            for bi, b0 in enumerate((0, 64)):
                nc.tensor.matmul(
                    pqk[:, bi * 512 + r:bi * 512 + 512],
                    st.kT[hp][b0:b0 + 64, kk * 128:(kk + 1) * 128],
                    st.qT[hp][b0:b0 + 64, tq0 + r:tq0 + 512],
                    start=True, stop=True)
            ptb = st.ptp.tile([128, 1024], bf16, name="ptb", tag="pt")
            if r == 0:
                nc.scalar.activation(ptb[:], pqk[:], AF.Exp)
            else:
                nc.scalar.activation(
                    ptb[:].rearrange("p (b w) -> p b w", b=2)[:, :, r:512],
                    pqk[:].rearrange("p (b w) -> p b w", b=2)[:, :, r:512],
                    AF.Exp)
            if kk >= 4 * j:
                nc.vector.tensor_tensor(
                    ptb[:].rearrange("p (b w) -> p b w", b=2)[:, :, r:r + 128],
                    ptb[:].rearrange("p (b w) -> p b w", b=2)[:, :, r:r + 128],
                    st.tri[:, None, :].to_broadcast((128, 2, 128)),
                    ALU.mult)
            for bi in range(2):
                h = 2 * hp + bi
                nc.tensor.matmul(
                    po[bi][0:65, r:512],
                    st.vn[kk][:, 65 * h:65 * h + 65],
                    ptb[:, bi * 512 + r:bi * 512 + 512],
                    start=(kk == 0), stop=(kk == nk - 1))
        for bi, b0 in enumerate((0, 64)):
            dr = aw.tile([1, 512], bf16, name="dr", tag="dr")
            nc.scalar.copy(dr[:], po[bi][64:65, :])
            pb = st.ps_aux((64, 512))
            nc.tensor.matmul(pb[:], st.onesr[:, 0:64], dr[:],
                             start=True, stop=True)
            rbi = aw.tile([64, 512], f32, name="rbi", tag="rbi")
            nc.vector.reciprocal_approx_fast(rbi[:], pb[:])
            tmp = aw.tile([64, 512], bf16, name="tmp", tag="tmp")
            nc.vector.tensor_tensor(tmp[:], po[bi][0:64, :], rbi[:], ALU.mult)
            asl = st.attA[hp][b0:b0 + 64, lc:lc + 512]
            if j < 2:
                nc.vector.tensor_scalar_mul(asl, tmp[:],
                                            st.sel_sb[0:64, sj:sj + 1])
            else:
                # stage at the same base partition as asl (tensor_tensor
                # requires equal base partitions for two SBUF inputs)
                tm2 = aw.tile([128, 512], bf16, name="tm2", tag="tm2")
                nc.vector.tensor_scalar_mul(tm2[b0:b0 + 64, :], tmp[:],
                                            st.sel_sb[0:64, sj:sj + 1])
                nc.vector.tensor_tensor(asl, asl, tm2[b0:b0 + 64, :], ALU.add)
            abc = aw.tile([64, 512], bf16, name="abc", tag="abc")
            nc.vector.tensor_scalar_mul(abc[:], tmp[:],
                                        st.seln_sb[0:64, sj:sj + 1])
            nc.sync.dma_start(
                st.rs_in[hp][sj, b0:b0 + 64, lc:lc + 512], abc[:])
        if j == 3:
            nc.gpsimd.collective_compute(
                "ReduceScatter", ALU.add, replica_groups=RG,
                ins=[st.rs_in[hp][:]], outs=[st.rs_out[hp][:]])


def _emit_proj(nc, st):
    """x2T = xresT + (att @ Wo)^T, transposed output [C, TH] in f32 (doubles
    as the FFN residual accumulator)."""
    att_sb = [st.prw.tile([128, TH], bf16, name=f"asb{k}", tag=f"asb{k}")
              for k in range(4)]
    for hp in range(4):
        nc.sync.dma_start(att_sb[hp][:], st.rs_out[hp][:])
    xr = [st.prw.tile([128, 512], f32, name="xr", tag=f"xr{i}") for i in range(4)]
    for tch in range(2):
        for cc in range(8):
            g = tch * 8 + cc
            if g % 4 == 0:
                for i in range(4):
                    ccx, tcx = (g + i) % 8, (g + i) // 8
                    nc.sync.dma_start(
                        xr[i][:], st.xrest_h[ccx * 128:(ccx + 1) * 128,
                                             tcx * 512:(tcx + 1) * 512])
            pg = st.ps_g()
            for k in range(4):
                nc.tensor.matmul(pg[:], st.wo_sb[k][:, cc * 128:(cc + 1) * 128],
                                 st.attA[k][:, tch * 512:(tch + 1) * 512],
                                 start=(k == 0), stop=False)
            for k in range(4):
                nc.tensor.matmul(pg[:], st.wo_sb[4 + k][:, cc * 128:(cc + 1) * 128],
                                 att_sb[k][:, tch * 512:(tch + 1) * 512],
                                 start=False, stop=(k == 3))
            nc.vector.tensor_tensor(st.x2T[g][:], pg[:], xr[g % 4][:], ALU.add)


def _emit_ln2_ffn(nc, st):
    """LN2 (column stats on x2T) + FFN with transposed output accumulated
    into x2T, then DMA out."""
    w = st.ffw
    h2T = [st.ffp.tile([128, 512], bf16, name="h2T", tag=f"h2T{i}")
           for i in range(16)]
    for tch in range(2):
        src = [st.x2T[tch * 8 + cc] for cc in range(8)]
        dst = [h2T[tch * 8 + cc] for cc in range(8)]
        _stats_and_norm(nc, st, src, 8, dst, "sq2", st.ffp, f32, st.onescolf)

    for g in range(4):
        w1g, w2g, utg = [], [], []
        for ff in range(8):
            f = g * 8 + ff
            w1c = w.tile([128, 1024], bf16, name="w1c", tag="w1c")
            nc.sync.dma_start(w1c[:], st.w1_h[f, :, :])
            w1g.append(w1c)
            w2t = w.tile([128, C], bf16, name="w2t", tag="w2t")
            nc.sync.dma_start(w2t[:], st.w2_h[f * 128:(f + 1) * 128, :])
            w2g.append(w2t)
        for ff in range(8):
            f = g * 8 + ff
            ut = w.tile([128, TH], bf16, name="ut", tag="ut")
            for tch in range(2):
                pg = st.ps_po(tch)
                for k in range(8):
                    nc.tensor.matmul(pg[:], w1g[ff][:, k * 128:(k + 1) * 128],
                                     h2T[tch * 8 + k][:],
                                     start=(k == 0), stop=(k == 7))
                nc.vector.tensor_scalar(
                    ut[:, tch * 512:(tch + 1) * 512], pg[:],
                    st.b1_sb[:, f:f + 1], 0.0, ALU.add, ALU.max)
            utg.append(ut)
        for tch in range(2):
            for cc in range(8):
                pg = st.ps_g()
                for ff in range(8):
                    nc.tensor.matmul(pg[:], w2g[ff][:, cc * 128:(cc + 1) * 128],
                                     utg[ff][:, tch * 512:(tch + 1) * 512],
                                     start=(ff == 0),
                                     stop=(ff == 7 and g > 0))
                if g == 0:
                    nc.tensor.matmul(pg[:], st.b2_sb[:, cc * 128:(cc + 1) * 128],
                                     st.onesr[:, 0:512], start=False, stop=True)
                gidx = tch * 8 + cc
                nc.vector.tensor_tensor(st.x2T[gidx][:], pg[:],
                                        st.x2T[gidx][:], ALU.add)
                if g == 3:
                    nc.sync.dma_start(
                        st.y_h[cc * 128:(cc + 1) * 128,
                               tch * 512:(tch + 1) * 512],
                        st.x2T[gidx][:])


def build_program():
    if "nc" in _CACHE:
        return _CACHE["nc"]
    nc = bacc.Bacc(None)
    st = S()

    st.xt_h = nc.declare_dram_parameter("xt", [C, T], bf16, isOutput=False)
    st.xrest_h = nc.declare_dram_parameter("xrest", [C, TH], f32, isOutput=False)
    st.wq_h = nc.declare_dram_parameter("wq", [C, H], bf16, isOutput=False)
    st.wk_h = nc.declare_dram_parameter("wk", [C, H], bf16, isOutput=False)
    st.wv_h = nc.declare_dram_parameter("wv", [C, H], bf16, isOutput=False)
    bq_h = nc.declare_dram_parameter("bq", [128, 4], f32, isOutput=False)
    bk_h = nc.declare_dram_parameter("bk", [128, 4], f32, isOutput=False)
    bv_h = nc.declare_dram_parameter("bv", [1, H], bf16, isOutput=False)
    st.wo_h = nc.declare_dram_parameter("wo", [C, C], bf16, isOutput=False)
    st.w1_h = nc.declare_dram_parameter("w1", [32, 128, 1024], bf16,
                                        isOutput=False)
    b1_h = nc.declare_dram_parameter("b1", [128, 32], f32, isOutput=False)
    st.w2_h = nc.declare_dram_parameter("w2", [F, C], bf16, isOutput=False)
    b2_h = nc.declare_dram_parameter("b2", [1, C], bf16, isOutput=False)
    tri_h = nc.declare_dram_parameter("tri", [128, 128], bf16, isOutput=False)
    onesr_h = nc.declare_dram_parameter("onesr", [1, 512], bf16, isOutput=False)
    onescol_h = nc.declare_dram_parameter("onescol", [128, 1], bf16,
                                          isOutput=False)
    onescolf_h = nc.declare_dram_parameter("onescolf", [128, 1], f32,
                                           isOutput=False)
    ones8_h = nc.declare_dram_parameter("ones8", [128, 8], bf16, isOutput=False)
    sel_h = nc.declare_dram_parameter("sel", [128, 2], f32, isOutput=False)
    seln_h = nc.declare_dram_parameter("seln", [128, 2], f32, isOutput=False)
    st.y_h = nc.declare_dram_parameter("y", [C, TH], f32, isOutput=True)

    st.rs_in = [nc.dram_tensor(f"rs_in{hp}", [2, 128, TH], bf16)
                for hp in range(4)]
    st.rs_out = [nc.dram_tensor(f"rs_out{hp}", [128, TH], bf16)
                 for hp in range(4)]

    with tile.TileContext(nc) as tc, ExitStack() as stack:
        st.tc = tc
        cst = stack.enter_context(tc.tile_pool(name="const", bufs=1))
        # PSUM pools: psA single-buffered (pqk 2 banks + poA + poB),
        # psB double-buffered (gemm + aux) -> 4 + 4 = 8 banks.
        psA = stack.enter_context(tc.tile_pool(name="psA", bufs=1, space="PSUM"))
        psB = stack.enter_context(tc.tile_pool(name="psB", bufs=2, space="PSUM"))
        st.work = stack.enter_context(tc.tile_pool(name="work", bufs=2))
        st.rows = stack.enter_context(tc.tile_pool(name="rows", bufs=1))

        def ps_g():
            return psB.tile([128, 512], f32, tag="gemm", name="ps_gemm")

        def ps_aux(shape):
            return psB.tile(list(shape), f32, tag="aux", name="ps_aux")

        def ps_qk():
            return psA.tile([128, 1024], f32, tag="pqk", name="ps_qk")

        def ps_po(i):
            return psA.tile([128, 512], f32, tag=f"po{i}", name=f"ps_po{i}")
        st.ps_g, st.ps_aux, st.ps_qk, st.ps_po = ps_g, ps_aux, ps_qk, ps_po

        st.tri = cst.tile([128, 128], bf16, name="tri")
        st.onesr = cst.tile([1, 512], bf16, name="onesr")
        st.onescol = cst.tile([128, 1], bf16, name="onescol")
        st.onescolf = cst.tile([128, 1], f32, name="onescolf")
        st.ones8 = cst.tile([128, 8], bf16, name="ones8")
        st.bq_sb = cst.tile([128, 4], f32, name="bq_sb")
        st.bk_sb = cst.tile([128, 4], f32, name="bk_sb")
        st.bv_sb = cst.tile([1, H], bf16, name="bv_sb")
        st.b1_sb = cst.tile([128, 32], f32, name="b1_sb")
        st.b2_sb = cst.tile([1, C], bf16, name="b2_sb")
        st.sel_sb = cst.tile([128, 2], f32, name="sel_sb")
        st.seln_sb = cst.tile([128, 2], f32, name="seln_sb")
        for t_, h_ in [(st.tri, tri_h), (st.onesr, onesr_h),
                       (st.onescol, onescol_h), (st.onescolf, onescolf_h),
                       (st.ones8, ones8_h),
                       (st.bq_sb, bq_h), (st.bk_sb, bk_h), (st.bv_sb, bv_h),
                       (st.b1_sb, b1_h), (st.b2_sb, b2_h),
                       (st.sel_sb, sel_h), (st.seln_sb, seln_h)]:
            nc.sync.dma_start(t_[:], h_[:])

        resid = stack.enter_context(tc.tile_pool(name="resid", bufs=1))
        st.x2T = [resid.tile([128, 512], f32, name=f"x2T{i}", tag=f"x2T{i}")
                  for i in range(16)]

        with tc.tile_pool(name="persist", bufs=1) as persist:
            st.qT = [persist.tile([128, T], bf16, name=f"qT{i}", tag=f"qT{i}")
                     for i in range(4)]
            st.kT = [persist.tile([128, T], bf16, name=f"kT{i}", tag=f"kT{i}")
                     for i in range(4)]
            st.vn = [persist.tile([128, 520], bf16, name=f"vn{i}", tag=f"vn{i}")
                     for i in range(16)]
            st.attA = [persist.tile([128, TH], bf16, name=f"attA{i}",
                                    tag=f"attA{i}") for i in range(4)]
            st.wo_sb = [persist.tile([128, C], bf16, name=f"wo{k}", tag=f"wo{k}")
                        for k in range(8)]
            for tt in range(16):
                nc.sync.dma_start(
                    st.vn[tt][:, 0:520].rearrange("p (h e) -> p h e",
                                                  h=8)[:, :, 64:65],
                    st.ones8[:].rearrange("p (h o) -> p h o", h=8))
            for k in range(8):
                nc.sync.dma_start(st.wo_sb[k][:],
                                  st.wo_h[k * 128:(k + 1) * 128, :])

            with tc.tile_pool(name="qkvw", bufs=1) as qkvw, \
                 tc.tile_pool(name="xh", bufs=2) as xh, \
                 tc.tile_pool(name="ptp", bufs=3) as ptp, \
                 tc.tile_pool(name="aw", bufs=2) as aw:
                st.wq_sb = [qkvw.tile([128, H], bf16, name=f"wq{k}",
                                      tag=f"wq{k}") for k in range(8)]
                st.wk_sb = [qkvw.tile([128, H], bf16, name=f"wk{k}",
                                      tag=f"wk{k}") for k in range(8)]
                st.wv_sb = [qkvw.tile([128, H], bf16, name=f"wv{k}",
                                      tag=f"wv{k}") for k in range(8)]
                st.xh, st.ptp, st.aw = xh, ptp, aw
                for k in range(8):
                    nc.sync.dma_start(st.wq_sb[k][:],
                                      st.wq_h[k * 128:(k + 1) * 128, :])
                    nc.sync.dma_start(st.wk_sb[k][:],
                                      st.wk_h[k * 128:(k + 1) * 128, :])
                    nc.sync.dma_start(st.wv_sb[k][:],
                                      st.wv_h[k * 128:(k + 1) * 128, :])

                for jc in range(4):
                    _emit_qkv_chunk(nc, st, jc)
                    _emit_attn_chunk(nc, st, jc)

            with tc.tile_pool(name="prw", bufs=1) as prw:
                st.prw = prw
                _emit_proj(nc, st)

        with tc.tile_pool(name="ffp", bufs=1) as ffp, \
             tc.tile_pool(name="ffw", bufs=18) as ffw:
            st.ffp, st.ffw = ffp, ffw
            _emit_ln2_ffn(nc, st)

    nc.compile()
    _CACHE["nc"] = nc
    return nc


def make_inputs(x, Wq, Wk, Wv, Wo, bo, W1, b1, W2, b2,
                ln1_g, ln1_b, ln2_g, ln2_b):
    """Build per-core input maps (host-side sharding, transposes, LN folding)."""
    x = np.asarray(x, np.float32)
    scale = float(C) ** -0.5

    wq_eff = ln1_g[:, None] * Wq
    wk_eff = ln1_g[:, None] * Wk * scale
    wv_eff = ln1_g[:, None] * Wv
    bq_full = ln1_b @ Wq
    bk_full = (ln1_b @ Wk) * scale
    bv_full = ln1_b @ Wv
    w1_eff = ln2_g[:, None] * W1
    b1_eff = b1 + ln2_b @ W1

    BF = ml_dtypes.bfloat16
    tri = np.triu(np.ones((128, 128), BF))
    onesr = np.ones((1, 512), BF)
    onescol = np.ones((128, 1), BF)
    ones8 = np.ones((128, 8), BF)
    # W1 pre-shuffled: w1s[f, c, k*128+j] = w1_eff[k*128+c, f*128+j]
    w1s = np.ascontiguousarray(
        w1_eff.reshape(8, 128, 32, 128).transpose(2, 1, 0, 3)
        .reshape(32, 128, 1024).astype(BF))
    w2c = np.ascontiguousarray(W2.astype(BF))
    b1c = np.ascontiguousarray(b1_eff.reshape(32, 128).T)
    b2c = np.ascontiguousarray(b2.reshape(1, C).astype(BF))

    in_maps = []
    for core in range(8):
        b, s = core // 2, core % 2
        cs = slice(s * H, (s + 1) * H)
        ts = slice(s * TH, (s + 1) * TH)
        own = np.arange(s * H, (s + 1) * H)
        other = np.arange((1 - s) * H, (2 - s) * H)
        perm = np.concatenate([own, other])
        in_maps.append({
            "xt": np.ascontiguousarray(x[b].T.astype(BF)),
            "xrest": np.ascontiguousarray((x[b, ts, :] + bo[None, :]).T),
            "wq": np.ascontiguousarray(wq_eff[:, cs].astype(BF)),
            "wk": np.ascontiguousarray(wk_eff[:, cs].astype(BF)),
            "wv": np.ascontiguousarray(wv_eff[:, cs].astype(BF)),
            "bq": np.ascontiguousarray(bq_full[cs].reshape(4, 128).T),
            "bk": np.ascontiguousarray(bk_full[cs].reshape(4, 128).T),
            "bv": np.ascontiguousarray(bv_full[cs].reshape(1, H).astype(BF)),
            "wo": np.ascontiguousarray(Wo[perm, :].astype(BF)),
            "w1": w1s, "b1": b1c, "w2": w2c, "b2": b2c,
            "tri": tri, "onesr": onesr, "onescol": onescol,
            "onescolf": np.ones((128, 1), np.float32), "ones8": ones8,
            "sel": np.tile(np.eye(2, dtype=np.float32)[s][None, :], (128, 1)),
            "seln": np.tile(np.eye(2, dtype=np.float32)[1 - s][None, :], (128, 1)),
        })
    return in_maps


def kernel(**inputs):
    nc = build_program()
    in_maps = make_inputs(**{k: np.asarray(v, np.float32)
                             for k, v in inputs.items()})
    res = run_bass_kernel_spmd(nc, in_maps, list(range(8)))
    out = np.empty((B, T, C), np.float32)
    for core in range(8):
        b, s = core // 2, core % 2
        out[b, s * TH:(s + 1) * TH, :] = res.results[core]["y"].T
    return out


# revision 34
# speedup vs baseline: 1.2345x; 1.1760x over previous
"""Trainium2 Bass kernel for a dense transformer block (B=4, T=2048, C=1024, 16 heads).

Sharding over 8 NeuronCores: core i handles batch b=i//2 with shard s=i%2.
 - LN1 + QKV + causal attention for its 8 heads (c-slice [512s, 512s+512)) over full T
 - exchange of attention outputs within the (b) pair via 4 per-head-pair
   ReduceScatter ops (zero-masked, SPMD-symmetric)
 - proj + LN2 + FFN + residuals on its t-half rows [1024s, 1024s+1024)

All activations flow in TRANSPOSED layout [C, T] (host passes xT / xresT and
un-transposes y), which removes every PE transpose.  LayerNorm is computed as
column statistics (partition-sum matmuls) + broadcast normalize on DVE.
LayerNorm gains/biases are folded into the weights on the host.
"""

from contextlib import ExitStack

import ml_dtypes
import numpy as np

import concourse.bass as bass
import concourse.mybir as mybir
import concourse.tile as tile
from concourse import bacc
from concourse.bass_utils import run_bass_kernel_spmd

f32 = mybir.dt.float32
bf16 = mybir.dt.bfloat16
AF = mybir.ActivationFunctionType
ALU = mybir.AluOpType
AX = mybir.AxisListType

B, T, C = 4, 2048, 1024
NH, D = 16, 64
F = 4 * C
H = C // 2            # per-core head c-slice (8 heads)
TH = T // 2           # per-core t-half for proj/FFN
NC_CHUNK = 512        # t-chunk for qkv/attention
EPS = 1e-5
RG = [[0, 1], [2, 3], [4, 5], [6, 7]]

_CACHE = {}


class S:
    pass


def _stats_and_norm(nc, st, src_tiles, n_src, dst_tiles, sq_tag, pool, sq_dt,
                    ones_lhs):
    """Column LayerNorm: src [128, 512] tiles covering 1024 c-rows -> dst bf16
    tiles (x - mu) * rsig per column.  sq_dt/ones_lhs match the src dtype so
    matmul operand dtypes agree."""
    work = st.work
    # squares on ACT (keeps DVE free for the copy-out path)
    sqs = []
    for k in range(n_src):
        sq = pool.tile([128, 512], sq_dt, name="sq", tag=f"{sq_tag}{k % 2}")
        nc.scalar.activation(sq[:], src_tiles[k][:], AF.Square)
        sqs.append(sq)
    # partition sums via M=1 matmuls; separate psum groups for sum and sumsq
    # (sharing a bank risks a fatal PE-write/DVE-read same-bank collision)
    pst = st.ps_aux((1, 512))
    psq = st.ps_aux((1, 512))
    for k in range(n_src):
        nc.tensor.matmul(pst[:], ones_lhs[:], src_tiles[k][:],
                         start=(k == 0), stop=(k == n_src - 1))
        nc.tensor.matmul(psq[:], ones_lhs[:], sqs[k][:],
                         start=(k == 0), stop=(k == n_src - 1))
    # row math (mu/sd rows in bf16 so the broadcast matmuls match onesr's dtype)
    mu = st.rows.tile([1, 512], bf16, name="mu_r", tag="mu_r")
    msq = st.rows.tile([1, 512], f32, name="msq_r", tag="msq_r")
    var = st.rows.tile([1, 512], f32, name="var_r", tag="var_r")
    sd = st.rows.tile([1, 512], bf16, name="sd_r", tag="sd_r")
    nc.vector.tensor_scalar_mul(mu[:], pst[:], 1.0 / C)
    nc.vector.tensor_scalar_mul(msq[:], psq[:], 1.0 / C)
    nc.vector.tensor_tensor(var[:], mu[:], mu[:], ALU.mult)
    nc.vector.tensor_tensor(var[:], msq[:], var[:], ALU.subtract)
    nc.vector.tensor_scalar_add(var[:], var[:], EPS)
    nc.scalar.activation(sd[:], var[:], AF.Sqrt)
    # broadcasts
    pmu = st.ps_aux((128, 512))
    nc.tensor.matmul(pmu[:], st.onesr[:, 0:128], mu[:], start=True, stop=True)
    mu_b = work.tile([128, 512], bf16, name="mu_b", tag="mu_b")
    nc.scalar.copy(mu_b[:], pmu[:])
    psd = st.ps_aux((128, 512))
    nc.tensor.matmul(psd[:], st.onesr[:, 0:128], sd[:], start=True, stop=True)
    rsig_b = work.tile([128, 512], f32, name="rsig_b", tag="rsig_b")
    nc.vector.reciprocal_approx_fast(rsig_b[:], psd[:])
    rsig_bb = work.tile([128, 512], bf16, name="rsig_bb", tag="rsig_bb")
    nc.scalar.copy(rsig_bb[:], rsig_b[:])
    # normalize (all-bf16 tensor_tensor gets the DVE 2x mode, no implicit casts)
    for k in range(n_src):
        dif = pool.tile([128, 512], bf16, name="dif", tag=f"{sq_tag}{k % 2}")
        nc.vector.tensor_tensor(dif[:], src_tiles[k][:], mu_b[:], ALU.subtract)
        nc.vector.tensor_tensor(dst_tiles[k][:], dif[:], rsig_bb[:], ALU.mult)


def _emit_qkv_chunk(nc, st, jc):
    """LN1 stats + q/k/v GEMMs for t-chunk jc (columns [jc*512, jc*512+512))."""
    xh, ps_g = st.xh, st.ps_g
    xt = []
    for k in range(8):
        t_ = xh.tile([128, 512], bf16, name="xt", tag=f"xt{k}")
        nc.sync.dma_start(t_[:], st.xt_h[k * 128:(k + 1) * 128,
                                          jc * 512:(jc + 1) * 512])
        xt.append(t_)
    hT = [xh.tile([128, 512], bf16, name="hT", tag=f"hT{k}") for k in range(8)]
    _stats_and_norm(nc, st, xt, 8, hT, "sq", xh, bf16, st.onescol)

    # q, k GEMMs -> transposed layout [c' 128, t 512]
    for dst, wsb, bsb in ((st.qT, st.wq_sb, st.bq_sb), (st.kT, st.wk_sb, st.bk_sb)):
        for co in range(4):
            pg = ps_g()
            for k in range(8):
                nc.tensor.matmul(pg[:], wsb[k][:, co * 128:(co + 1) * 128],
                                 hT[k][:], start=(k == 0), stop=(k == 7))
            nc.scalar.activation(dst[co][:, jc * 512:(jc + 1) * 512], pg[:],
                                 AF.Identity, bias=bsb[:, co:co + 1])
    # v GEMM -> natural layout [t 128, cv 512], strided into vn (65-col heads)
    for tt4 in range(4):
        tt = jc * 4 + tt4
        pg = ps_g()
        for k in range(8):
            nc.tensor.matmul(pg[:], hT[k][:, tt4 * 128:(tt4 + 1) * 128],
                             st.wv_sb[k][:], start=(k == 0), stop=False)
        nc.tensor.matmul(pg[:], st.onesr[:, 0:128], st.bv_sb[:],
                         start=False, stop=True)
        nc.scalar.copy(
            st.vn[tt][:, 0:520].rearrange("p (h e) -> p h e", h=8)[:, :, 0:64],
            pg[:].rearrange("p (h d) -> p h d", h=8))


def _emit_attn_chunk(nc, st, j):
    """Causal attention of q-chunk j against k-chunks 0..j, all 4 head-pairs.
    Writes the own-t-half part into compact attA and DMAs the peer part into
    rs_in; fires the per-hp ReduceScatter after the last chunk."""
    aw = st.aw
    tq0 = j * 512
    nk = 4 * (j + 1)
    sj = j // 2        # which t-half this q-chunk belongs to
    lc = (j % 2) * 512  # column inside the compact [128, TH] buffers
    for hp in range(4):
        po = [st.ps_po(0), st.ps_po(1)]
        for kk in range(nk):
            r = 128 * (kk - 4 * j) if kk >= 4 * j else 0
            pqk = st.ps_qk()
            for bi, b0 in enumerate((0, 64)):
                nc.tensor.matmul(
                    pqk[:, bi * 512 + r:bi * 512 + 512],
                    st.kT[hp][b0:b0 + 64, kk * 128:(kk + 1) * 128],
                    st.qT[hp][b0:b0 + 64, tq0 + r:tq0 + 512],
                    start=True, stop=True)
            ptb = st.ptp.tile([128, 1024], bf16, name="ptb", tag="pt")
            if r == 0:
                nc.scalar.activation(ptb[:], pqk[:], AF.Exp)
            else:
                nc.scalar.activation(
                    ptb[:].rearrange("p (b w) -> p b w", b=2)[:, :, r:512],
                    pqk[:].rearrange("p (b w) -> p b w", b=2)[:, :, r:512],
                    AF.Exp)
            if kk >= 4 * j:
                nc.vector.tensor_tensor(
                    ptb[:].rearrange("p (b w) -> p b w", b=2)[:, :, r:r + 128],
                    ptb[:].rearrange("p (b w) -> p b w", b=2)[:, :, r:r + 128],
                    st.tri[:, None, :].to_broadcast((128, 2, 128)),
                    ALU.mult)
            for bi in range(2):
                h = 2 * hp + bi
                nc.tensor.matmul(
                    po[bi][0:65, r:512],
                    st.vn[kk][:, 65 * h:65 * h + 65],
                    ptb[:, bi * 512 + r:bi * 512 + 512],
                    start=(kk == 0), stop=(kk == nk - 1))
        for bi, b0 in enumerate((0, 64)):
            dr = aw.tile([1, 512], bf16, name="dr", tag="dr")
            nc.scalar.copy(dr[:], po[bi][64:65, :])
            pb = st.ps_g((64, 512))
            nc.tensor.matmul(pb[:], st.onesr[:, 0:64], dr[:],
                             start=True, stop=True)
            rbi = aw.tile([64, 512], f32, name="rbi", tag="rbi")
            nc.vector.reciprocal_approx_fast(rbi[:], pb[:])
            tmp = aw.tile([64, 512], bf16, name="tmp", tag="tmp")
            nc.vector.tensor_tensor(tmp[:], po[bi][0:64, :], rbi[:], ALU.mult)
            asl = st.attA[hp][b0:b0 + 64, lc:lc + 512]
            if j < 2:
                nc.vector.tensor_scalar_mul(asl, tmp[:],
                                            st.sel_sb[0:64, sj:sj + 1])
            else:
                # stage at the same base partition as asl (tensor_tensor
                # requires equal base partitions for two SBUF inputs)
                tm2 = aw.tile([128, 512], bf16, name="tm2", tag="tm2")
                nc.vector.tensor_scalar_mul(tm2[b0:b0 + 64, :], tmp[:],
                                            st.sel_sb[0:64, sj:sj + 1])
                nc.vector.tensor_tensor(asl, asl, tm2[b0:b0 + 64, :], ALU.add)
            abc = aw.tile([64, 512], bf16, name="abc", tag="abc")
            nc.vector.tensor_scalar_mul(abc[:], tmp[:],
                                        st.seln_sb[0:64, sj:sj + 1])
            nc.sync.dma_start(
                st.rs_in[hp][sj, b0:b0 + 64, lc:lc + 512], abc[:])
        if j == 3:
            nc.gpsimd.collective_compute(
                "ReduceScatter", ALU.add, replica_groups=RG,
                ins=[st.rs_in[hp][:]], outs=[st.rs_out[hp][:]])


def _emit_proj(nc, st):
    """x2T = xresT + (att @ Wo)^T, transposed output [C, TH] in f32 (doubles
    as the FFN residual accumulator)."""
    att_sb = [st.prw.tile([128, TH], bf16, name=f"asb{k}", tag=f"asb{k}")
              for k in range(4)]
    for hp in range(4):
        nc.sync.dma_start(att_sb[hp][:], st.rs_out[hp][:])
    xr = [st.prw.tile([128, 512], f32, name="xr", tag=f"xr{i}") for i in range(4)]
    for tch in range(2):
        for cc in range(8):
            g = tch * 8 + cc
            if g % 4 == 0:
                for i in range(4):
                    ccx, tcx = (g + i) % 8, (g + i) // 8
                    nc.sync.dma_start(
                        xr[i][:], st.xrest_h[ccx * 128:(ccx + 1) * 128,
                                             tcx * 512:(tcx + 1) * 512])
            pg = st.ps_g()
            for k in range(4):
                nc.tensor.matmul(pg[:], st.wo_sb[k][:, cc * 128:(cc + 1) * 128],
                                 st.attA[k][:, tch * 512:(tch + 1) * 512],
                                 start=(k == 0), stop=False)
            for k in range(4):
                nc.tensor.matmul(pg[:], st.wo_sb[4 + k][:, cc * 128:(cc + 1) * 128],
                                 att_sb[k][:, tch * 512:(tch + 1) * 512],
                                 start=False, stop=(k == 3))
            nc.vector.tensor_tensor(st.x2T[g][:], pg[:], xr[g % 4][:], ALU.add)


def _emit_ln2_ffn(nc, st):
    """LN2 (column stats on x2T) + FFN with transposed output accumulated
    into x2T, then DMA out."""
    w = st.ffw
    h2T = [st.ffp.tile([128, 512], bf16, name="h2T", tag=f"h2T{i}")
           for i in range(16)]
    for tch in range(2):
        src = [st.x2T[tch * 8 + cc] for cc in range(8)]
        dst = [h2T[tch * 8 + cc] for cc in range(8)]
        _stats_and_norm(nc, st, src, 8, dst, "sq2", st.ffp, f32, st.onescolf)

    for g in range(4):
        w1g, w2g, utg = [], [], []
        for ff in range(8):
            f = g * 8 + ff
            w1c = w.tile([128, 1024], bf16, name="w1c", tag="w1c")
            nc.sync.dma_start(w1c[:], st.w1_h[f, :, :])
            w1g.append(w1c)
            w2t = w.tile([128, C], bf16, name="w2t", tag="w2t")
            nc.sync.dma_start(w2t[:], st.w2_h[f * 128:(f + 1) * 128, :])
            w2g.append(w2t)
        for ff in range(8):
            f = g * 8 + ff
            ut = w.tile([128, TH], bf16, name="ut", tag="ut")
            for tch in range(2):
                pg = st.ps_po(tch)
                for k in range(8):
                    nc.tensor.matmul(pg[:], w1g[ff][:, k * 128:(k + 1) * 128],
                                     h2T[tch * 8 + k][:],
                                     start=(k == 0), stop=(k == 7))
                nc.vector.tensor_scalar(
                    ut[:, tch * 512:(tch + 1) * 512], pg[:],
                    st.b1_sb[:, f:f + 1], 0.0, ALU.add, ALU.max)
            utg.append(ut)
        for tch in range(2):
            for cc in range(8):
                pg = st.ps_g()
                for ff in range(8):
                    nc.tensor.matmul(pg[:], w2g[ff][:, cc * 128:(cc + 1) * 128],
                                     utg[ff][:, tch * 512:(tch + 1) * 512],
                                     start=(ff == 0),
                                     stop=(ff == 7 and g > 0))
                if g == 0:
                    nc.tensor.matmul(pg[:], st.b2_sb[:, cc * 128:(cc + 1) * 128],
                                     st.onesr[:, 0:512], start=False, stop=True)
                gidx = tch * 8 + cc
                nc.vector.tensor_tensor(st.x2T[gidx][:], pg[:],
                                        st.x2T[gidx][:], ALU.add)
                if g == 3:
                    nc.sync.dma_start(
                        st.y_h[cc * 128:(cc + 1) * 128,
                               tch * 512:(tch + 1) * 512],
                        st.x2T[gidx][:])


def build_program():
    if "nc" in _CACHE:
        return _CACHE["nc"]
    nc = bacc.Bacc(None)
    st = S()

    st.xt_h = nc.declare_dram_parameter("xt", [C, T], bf16, isOutput=False)
    st.xrest_h = nc.declare_dram_parameter("xrest", [C, TH], f32, isOutput=False)
    st.wq_h = nc.declare_dram_parameter("wq", [C, H], bf16, isOutput=False)
    st.wk_h = nc.declare_dram_parameter("wk", [C, H], bf16, isOutput=False)
    st.wv_h = nc.declare_dram_parameter("wv", [C, H], bf16, isOutput=False)
    bq_h = nc.declare_dram_parameter("bq", [128, 4], f32, isOutput=False)
    bk_h = nc.declare_dram_parameter("bk", [128, 4], f32, isOutput=False)
    bv_h = nc.declare_dram_parameter("bv", [1, H], bf16, isOutput=False)
    st.wo_h = nc.declare_dram_parameter("wo", [C, C], bf16, isOutput=False)
    st.w1_h = nc.declare_dram_parameter("w1", [32, 128, 1024], bf16,
                                        isOutput=False)
    b1_h = nc.declare_dram_parameter("b1", [128, 32], f32, isOutput=False)
    st.w2_h = nc.declare_dram_parameter("w2", [F, C], bf16, isOutput=False)
    b2_h = nc.declare_dram_parameter("b2", [1, C], bf16, isOutput=False)
    tri_h = nc.declare_dram_parameter("tri", [128, 128], bf16, isOutput=False)
    onesr_h = nc.declare_dram_parameter("onesr", [1, 512], bf16, isOutput=False)
    onescol_h = nc.declare_dram_parameter("onescol", [128, 1], bf16,
                                          isOutput=False)
    onescolf_h = nc.declare_dram_parameter("onescolf", [128, 1], f32,
                                           isOutput=False)
    ones8_h = nc.declare_dram_parameter("ones8", [128, 8], bf16, isOutput=False)
    sel_h = nc.declare_dram_parameter("sel", [128, 2], f32, isOutput=False)
    seln_h = nc.declare_dram_parameter("seln", [128, 2], f32, isOutput=False)
    st.y_h = nc.declare_dram_parameter("y", [C, TH], f32, isOutput=True)

    st.rs_in = [nc.dram_tensor(f"rs_in{hp}", [2, 128, TH], bf16)
                for hp in range(4)]
    st.rs_out = [nc.dram_tensor(f"rs_out{hp}", [128, TH], bf16)
                 for hp in range(4)]

    with tile.TileContext(nc) as tc, ExitStack() as stack:
        st.tc = tc
        cst = stack.enter_context(tc.tile_pool(name="const", bufs=1))
        # PSUM pools: psA single-buffered (pqk 2 banks + poA + poB),
        # psB double-buffered (gemm + aux) -> 4 + 4 = 8 banks.
        psA = stack.enter_context(tc.tile_pool(name="psA", bufs=1, space="PSUM"))
        psB = stack.enter_context(tc.tile_pool(name="psB", bufs=2, space="PSUM"))
        st.work = stack.enter_context(tc.tile_pool(name="work", bufs=2))
        st.rows = stack.enter_context(tc.tile_pool(name="rows", bufs=1))

        def ps_g(shape=(128, 512)):
            return psB.tile(list(shape), f32, tag="gemm", name="ps_gemm")

        def ps_aux(shape):
            return psB.tile(list(shape), f32, tag="aux", name="ps_aux")

        def ps_qk():
            return psA.tile([128, 1024], f32, tag="pqk", name="ps_qk")

        def ps_po(i):
            return psA.tile([128, 512], f32, tag=f"po{i}", name=f"ps_po{i}")
        st.ps_g, st.ps_aux, st.ps_qk, st.ps_po = ps_g, ps_aux, ps_qk, ps_po

        st.tri = cst.tile([128, 128], bf16, name="tri")
        st.onesr = cst.tile([1, 512], bf16, name="onesr")
        st.onescol = cst.tile([128, 1], bf16, name="onescol")
        st.onescolf = cst.tile([128, 1], f32, name="onescolf")
        st.ones8 = cst.tile([128, 8], bf16, name="ones8")
        st.bq_sb = cst.tile([128, 4], f32, name="bq_sb")
        st.bk_sb = cst.tile([128, 4], f32, name="bk_sb")
        st.bv_sb = cst.tile([1, H], bf16, name="bv_sb")
        st.b1_sb = cst.tile([128, 32], f32, name="b1_sb")
        st.b2_sb = cst.tile([1, C], bf16, name="b2_sb")
        st.sel_sb = cst.tile([128, 2], f32, name="sel_sb")
        st.seln_sb = cst.tile([128, 2], f32, name="seln_sb")
        for t_, h_ in [(st.tri, tri_h), (st.onesr, onesr_h),
                       (st.onescol, onescol_h), (st.onescolf, onescolf_h),
                       (st.ones8, ones8_h),
                       (st.bq_sb, bq_h), (st.bk_sb, bk_h), (st.bv_sb, bv_h),
                       (st.b1_sb, b1_h), (st.b2_sb, b2_h),
                       (st.sel_sb, sel_h), (st.seln_sb, seln_h)]:
            nc.sync.dma_start(t_[:], h_[:])

        resid = stack.enter_context(tc.tile_pool(name="resid", bufs=1))
        st.x2T = [resid.tile([128, 512], f32, name=f"x2T{i}", tag=f"x2T{i}")
                  for i in range(16)]

        with tc.tile_pool(name="persist", bufs=1) as persist:
            st.qT = [persist.tile([128, T], bf16, name=f"qT{i}", tag=f"qT{i}")
                     for i in range(4)]
            st.kT = [persist.tile([128, T], bf16, name=f"kT{i}", tag=f"kT{i}")
                     for i in range(4)]
            st.vn = [persist.tile([128, 520], bf16, name=f"vn{i}", tag=f"vn{i}")
                     for i in range(16)]
            st.attA = [persist.tile([128, TH], bf16, name=f"attA{i}",
                                    tag=f"attA{i}") for i in range(4)]
            st.wo_sb = [persist.tile([128, C], bf16, name=f"wo{k}", tag=f"wo{k}")
                        for k in range(8)]
            for tt in range(16):
                nc.sync.dma_start(
                    st.vn[tt][:, 0:520].rearrange("p (h e) -> p h e",
                                                  h=8)[:, :, 64:65],
                    st.ones8[:].rearrange("p (h o) -> p h o", h=8))
            for k in range(8):
                nc.sync.dma_start(st.wo_sb[k][:],
                                  st.wo_h[k * 128:(k + 1) * 128, :])

            with tc.tile_pool(name="qkvw", bufs=1) as qkvw, \
                 tc.tile_pool(name="xh", bufs=2) as xh, \
                 tc.tile_pool(name="ptp", bufs=3) as ptp, \
                 tc.tile_pool(name="aw", bufs=2) as aw:
                st.wq_sb = [qkvw.tile([128, H], bf16, name=f"wq{k}",
                                      tag=f"wq{k}") for k in range(8)]
                st.wk_sb = [qkvw.tile([128, H], bf16, name=f"wk{k}",
                                      tag=f"wk{k}") for k in range(8)]
                st.wv_sb = [qkvw.tile([128, H], bf16, name=f"wv{k}",
                                      tag=f"wv{k}") for k in range(8)]
                st.xh, st.ptp, st.aw = xh, ptp, aw
                for k in range(8):
                    nc.sync.dma_start(st.wq_sb[k][:],
                                      st.wq_h[k * 128:(k + 1) * 128, :])
                    nc.sync.dma_start(st.wk_sb[k][:],
                                      st.wk_h[k * 128:(k + 1) * 128, :])
                    nc.sync.dma_start(st.wv_sb[k][:],
                                      st.wv_h[k * 128:(k + 1) * 128, :])

                for jc in range(4):
                    _emit_qkv_chunk(nc, st, jc)
                    _emit_attn_chunk(nc, st, jc)

            with tc.tile_pool(name="prw", bufs=1) as prw:
                st.prw = prw
                _emit_proj(nc, st)

        with tc.tile_pool(name="ffp", bufs=1) as ffp, \
             tc.tile_pool(name="ffw", bufs=18) as ffw:
            st.ffp, st.ffw = ffp, ffw
            _emit_ln2_ffn(nc, st)

    nc.compile()
    _CACHE["nc"] = nc
    return nc


def make_inputs(x, Wq, Wk, Wv, Wo, bo, W1, b1, W2, b2,
                ln1_g, ln1_b, ln2_g, ln2_b):
    """Build per-core input maps (host-side sharding, transposes, LN folding)."""
    x = np.asarray(x, np.float32)
    scale = float(C) ** -0.5

    wq_eff = ln1_g[:, None] * Wq
    wk_eff = ln1_g[:, None] * Wk * scale
    wv_eff = ln1_g[:, None] * Wv
    bq_full = ln1_b @ Wq
    bk_full = (ln1_b @ Wk) * scale
    bv_full = ln1_b @ Wv
    w1_eff = ln2_g[:, None] * W1
    b1_eff = b1 + ln2_b @ W1

    BF = ml_dtypes.bfloat16
    tri = np.triu(np.ones((128, 128), BF))
    onesr = np.ones((1, 512), BF)
    onescol = np.ones((128, 1), BF)
    ones8 = np.ones((128, 8), BF)
    # W1 pre-shuffled: w1s[f, c, k*128+j] = w1_eff[k*128+c, f*128+j]
    w1s = np.ascontiguousarray(
        w1_eff.reshape(8, 128, 32, 128).transpose(2, 1, 0, 3)
        .reshape(32, 128, 1024).astype(BF))
    w2c = np.ascontiguousarray(W2.astype(BF))
    b1c = np.ascontiguousarray(b1_eff.reshape(32, 128).T)
    b2c = np.ascontiguousarray(b2.reshape(1, C).astype(BF))

    in_maps = []
    for core in range(8):
        b, s = core // 2, core % 2
        cs = slice(s * H, (s + 1) * H)
        ts = slice(s * TH, (s + 1) * TH)
        own = np.arange(s * H, (s + 1) * H)
        other = np.arange((1 - s) * H, (2 - s) * H)
        perm = np.concatenate([own, other])
        in_maps.append({
            "xt": np.ascontiguousarray(x[b].T.astype(BF)),
            "xrest": np.ascontiguousarray((x[b, ts, :] + bo[None, :]).T),
            "wq": np.ascontiguousarray(wq_eff[:, cs].astype(BF)),
            "wk": np.ascontiguousarray(wk_eff[:, cs].astype(BF)),
            "wv": np.ascontiguousarray(wv_eff[:, cs].astype(BF)),
            "bq": np.ascontiguousarray(bq_full[cs].reshape(4, 128).T),
            "bk": np.ascontiguousarray(bk_full[cs].reshape(4, 128).T),
            "bv": np.ascontiguousarray(bv_full[cs].reshape(1, H).astype(BF)),
            "wo": np.ascontiguousarray(Wo[perm, :].astype(BF)),
            "w1": w1s, "b1": b1c, "w2": w2c, "b2": b2c,
            "tri": tri, "onesr": onesr, "onescol": onescol,
            "onescolf": np.ones((128, 1), np.float32), "ones8": ones8,
            "sel": np.tile(np.eye(2, dtype=np.float32)[s][None, :], (128, 1)),
            "seln": np.tile(np.eye(2, dtype=np.float32)[1 - s][None, :], (128, 1)),
        })
    return in_maps


def kernel(**inputs):
    nc = build_program()
    in_maps = make_inputs(**{k: np.asarray(v, np.float32)
                             for k, v in inputs.items()})
    res = run_bass_kernel_spmd(nc, in_maps, list(range(8)))
    out = np.empty((B, T, C), np.float32)
    for core in range(8):
        b, s = core // 2, core % 2
        out[b, s * TH:(s + 1) * TH, :] = res.results[core]["y"].T
    return out


# revision 45
# speedup vs baseline: 1.3655x; 1.1062x over previous
"""Trainium2 Bass kernel for a dense transformer block (B=4, T=2048, C=1024, 16 heads).

Sharding over 8 NeuronCores: core i handles batch b=i//2 with shard s=i%2.
 - LN1 + QKV + causal attention for its 8 heads (c-slice [512s, 512s+512)) over full T
 - exchange of attention outputs within the (b) pair via 4 per-head-pair
   ReduceScatter ops (zero-masked, SPMD-symmetric)
 - proj + LN2 + FFN + residuals on its t-half rows [1024s, 1024s+1024)

All activations flow in TRANSPOSED layout [C, T] (host passes xT / xresT and
un-transposes y), which removes every PE transpose.  LayerNorm is computed as
column statistics (partition-sum matmuls) + broadcast normalize on DVE.
LayerNorm gains/biases are folded into the weights on the host.
"""

from contextlib import ExitStack

import ml_dtypes
import numpy as np

import concourse.bass as bass
import concourse.mybir as mybir
import concourse.tile as tile
from concourse import bacc
from concourse.bass_utils import run_bass_kernel_spmd

f32 = mybir.dt.float32
bf16 = mybir.dt.bfloat16
AF = mybir.ActivationFunctionType
ALU = mybir.AluOpType
AX = mybir.AxisListType

B, T, C = 4, 2048, 1024
NH, D = 16, 64
F = 4 * C
H = C // 2            # per-core head c-slice (8 heads)
TH = T // 2           # per-core t-half for proj/FFN
NC_CHUNK = 512        # t-chunk for qkv/attention
EPS = 1e-5
RG = [[0, 1], [2, 3], [4, 5], [6, 7]]

_CACHE = {}


class S:
    pass


def _stats_and_norm(nc, st, src_tiles, n_src, dst_tiles, sq_tag, pool, sq_dt,
                    ones_lhs, sq_act=False):
    """Column LayerNorm: src [128, 512] tiles covering 1024 c-rows -> dst bf16
    tiles (x - mu) * rsig per column.  sq_dt/ones_lhs match the src dtype so
    matmul operand dtypes agree.  sq_act routes squares to ACT (use only when
    ACT is not busy with attention exp)."""
    work = st.work
    sqs = []
    for k in range(n_src):
        sq = pool.tile([128, 512], sq_dt, name="sq", tag=f"{sq_tag}{k % 2}")
        if sq_act:
            nc.scalar.activation(sq[:], src_tiles[k][:], AF.Square)
        else:
            nc.vector.tensor_tensor(sq[:], src_tiles[k][:], src_tiles[k][:],
                                    ALU.mult)
        sqs.append(sq)
    # partition sums via M=1 matmuls; separate psum groups for sum and sumsq
    # (sharing a bank risks a fatal PE-write/DVE-read same-bank collision)
    pst = st.ps_aux((1, 512))
    psq = st.ps_aux((1, 512))
    for k in range(n_src):
        nc.tensor.matmul(pst[:], ones_lhs[:], src_tiles[k][:],
                         start=(k == 0), stop=(k == n_src - 1))
        nc.tensor.matmul(psq[:], ones_lhs[:], sqs[k][:],
                         start=(k == 0), stop=(k == n_src - 1))
    # row math (mu/sd rows in bf16 so the broadcast matmuls match onesr's dtype)
    mu = st.rows.tile([1, 512], bf16, name="mu_r", tag="mu_r")
    msq = st.rows.tile([1, 512], f32, name="msq_r", tag="msq_r")
    var = st.rows.tile([1, 512], f32, name="var_r", tag="var_r")
    sd = st.rows.tile([1, 512], bf16, name="sd_r", tag="sd_r")
    nc.vector.tensor_scalar_mul(mu[:], pst[:], 1.0 / C)
    nc.vector.tensor_scalar_mul(msq[:], psq[:], 1.0 / C)
    nc.vector.tensor_tensor(var[:], mu[:], mu[:], ALU.mult)
    nc.vector.tensor_tensor(var[:], msq[:], var[:], ALU.subtract)
    nc.vector.tensor_scalar_add(var[:], var[:], EPS)
    nc.scalar.activation(sd[:], var[:], AF.Sqrt)
    # broadcasts
    pmu = st.ps_aux((128, 512))
    nc.tensor.matmul(pmu[:], st.onesr[:, 0:128], mu[:], start=True, stop=True)
    mu_b = work.tile([128, 512], bf16, name="mu_b", tag="mu_b")
    nc.vector.tensor_copy(out=mu_b[:], in_=pmu[:])
    psd = st.ps_aux((128, 512))
    nc.tensor.matmul(psd[:], st.onesr[:, 0:128], sd[:], start=True, stop=True)
    rsig_b = work.tile([128, 512], f32, name="rsig_b", tag="rsig_b")
    nc.vector.reciprocal_approx_fast(rsig_b[:], psd[:])
    rsig_bb = work.tile([128, 512], bf16, name="rsig_bb", tag="rsig_bb")
    nc.vector.tensor_copy(out=rsig_bb[:], in_=rsig_b[:])
    # normalize (all-bf16 tensor_tensor gets the DVE 2x mode, no implicit casts)
    for k in range(n_src):
        dif = pool.tile([128, 512], bf16, name="dif", tag=f"{sq_tag}{k % 2}")
        nc.vector.tensor_tensor(dif[:], src_tiles[k][:], mu_b[:], ALU.subtract)
        nc.vector.tensor_tensor(dst_tiles[k][:], dif[:], rsig_bb[:], ALU.mult)


def _dma_xt_chunk(nc, st, jc):
    xt = []
    for k in range(8):
        t_ = st.xh.tile([128, 512], bf16, name="xt", tag=f"xt{k}")
        nc.sync.dma_start(t_[:], st.xt_h[k * 128:(k + 1) * 128,
                                          jc * 512:(jc + 1) * 512])
        xt.append(t_)
    return xt


def _emit_qkv_chunk(nc, st, jc, xt_pre=None):
    """LN1 stats + q/k/v GEMMs for t-chunk jc (columns [jc*512, jc*512+512))."""
    xh, ps_g = st.xh, st.ps_g
    xt = xt_pre if xt_pre is not None else _dma_xt_chunk(nc, st, jc)
    hT = [xh.tile([128, 512], bf16, name="hT", tag=f"hT{k}") for k in range(8)]
    _stats_and_norm(nc, st, xt, 8, hT, "sq", xh, bf16, st.onescol)

    # q, k GEMMs -> transposed layout [c' 128, t 512]
    for dst, wsb, bsb in ((st.qT, st.wq_sb, st.bq_sb), (st.kT, st.wk_sb, st.bk_sb)):
        for co in range(4):
            pg = ps_g()
            for k in range(8):
                nc.tensor.matmul(pg[:], wsb[k][:, co * 128:(co + 1) * 128],
                                 hT[k][:], start=(k == 0), stop=(k == 7))
            nc.vector.tensor_scalar_add(dst[co][:, jc * 512:(jc + 1) * 512],
                                        pg[:], bsb[:, co:co + 1])
    # v GEMM -> natural layout [t 128, cv 512], strided into vn (65-col heads)
    for tt4 in range(4):
        tt = jc * 4 + tt4
        pg = ps_g()
        for k in range(8):
            nc.tensor.matmul(pg[:], hT[k][:, tt4 * 128:(tt4 + 1) * 128],
                             st.wv_sb[k][:], start=(k == 0), stop=False)
        nc.tensor.matmul(pg[:], st.onesr[:, 0:128], st.bv_sb[:],
                         start=False, stop=True)
        nc.vector.tensor_copy(
            out=st.vn[tt][:, 0:520].rearrange("p (h e) -> p h e", h=8)[:, :, 0:64],
            in_=pg[:].rearrange("p (h d) -> p h d", h=8))


def _emit_attn_chunk(nc, st, j):
    """Causal attention of q-chunk j against k-chunks 0..j, all 4 head-pairs.
    Writes the own-t-half part into compact attA and DMAs the peer part into
    rs_in; fires the per-hp ReduceScatter after the last chunk."""
    aw = st.aw
    tq0 = j * 512
    nk = 4 * (j + 1)
    sj = j // 2        # which t-half this q-chunk belongs to
    lc = (j % 2) * 512  # column inside the compact [128, TH] buffers
    for hp in range(4):
        po = [st.ps_po(0), st.ps_po(1)]
        for kk in range(nk):
            r = 128 * (kk - 4 * j) if kk >= 4 * j else 0
            pqk = st.ps_qk()
            for bi, b0 in enumerate((0, 64)):
                nc.tensor.matmul(
                    pqk[:, bi * 512 + r:bi * 512 + 512],
                    st.kT[hp][b0:b0 + 64, kk * 128:(kk + 1) * 128],
                    st.qT[hp][b0:b0 + 64, tq0 + r:tq0 + 512],
                    start=True, stop=True)
            ptb = st.ptp.tile([128, 1024], bf16, name="ptb", tag="pt")
            if r == 0:
                nc.scalar.activation(ptb[:], pqk[:], AF.Exp)
            else:
                nc.scalar.activation(
                    ptb[:].rearrange("p (b w) -> p b w", b=2)[:, :, r:512],
                    pqk[:].rearrange("p (b w) -> p b w", b=2)[:, :, r:512],
                    AF.Exp)
            if kk >= 4 * j:
                nc.vector.tensor_tensor(
                    ptb[:].rearrange("p (b w) -> p b w", b=2)[:, :, r:r + 128],
                    ptb[:].rearrange("p (b w) -> p b w", b=2)[:, :, r:r + 128],
                    st.tri[:, None, :].to_broadcast((128, 2, 128)),
                    ALU.mult)
            for bi in range(2):
                h = 2 * hp + bi
                nc.tensor.matmul(
                    po[bi][0:65, r:512],
                    st.vn[kk][:, 65 * h:65 * h + 65],
                    ptb[:, bi * 512 + r:bi * 512 + 512],
                    start=(kk == 0), stop=(kk == nk - 1))
        for bi, b0 in enumerate((0, 64)):
            dr = aw.tile([1, 512], bf16, name="dr", tag="dr")
            nc.scalar.copy(dr[:], po[bi][64:65, :])
            pb = st.ps_g((64, 512))
            nc.tensor.matmul(pb[:], st.onesr[:, 0:64], dr[:],
                             start=True, stop=True)
            rbi = aw.tile([64, 512], f32, name="rbi", tag="rbi")
            nc.vector.reciprocal_approx_fast(rbi[:], pb[:])
            tmp = aw.tile([64, 512], bf16, name="tmp", tag="tmp")
            nc.vector.tensor_tensor(tmp[:], po[bi][0:64, :], rbi[:], ALU.mult)
            asl = st.attA[hp][b0:b0 + 64, lc:lc + 512]
            if j < 2:
                nc.vector.tensor_scalar_mul(asl, tmp[:],
                                            st.sel_sb[0:64, sj:sj + 1])
            else:
                # stage at the same base partition as asl (tensor_tensor
                # requires equal base partitions for two SBUF inputs)
                tm2 = aw.tile([128, 512], bf16, name="tm2", tag="tm2")
                nc.vector.tensor_scalar_mul(tm2[b0:b0 + 64, :], tmp[:],
                                            st.sel_sb[0:64, sj:sj + 1])
                nc.vector.tensor_tensor(asl, asl, tm2[b0:b0 + 64, :], ALU.add)
            abc = aw.tile([64, 512], bf16, name="abc", tag="abc")
            nc.vector.tensor_scalar_mul(abc[:], tmp[:],
                                        st.seln_sb[0:64, sj:sj + 1])
            nc.sync.dma_start(
                st.rs_in[hp][sj, b0:b0 + 64, lc:lc + 512], abc[:])
        if j == 3:
            nc.gpsimd.collective_compute(
                "ReduceScatter", ALU.add, replica_groups=RG,
                ins=[st.rs_in[hp][:]], outs=[st.rs_out[hp][:]])


def _emit_proj(nc, st):
    """x2T = xresT + (att @ Wo)^T, transposed output [C, TH] in f32 (doubles
    as the FFN residual accumulator)."""
    att_sb = [st.prw.tile([128, TH], bf16, name=f"asb{k}", tag=f"asb{k}")
              for k in range(4)]
    for hp in range(4):
        nc.sync.dma_start(att_sb[hp][:], st.rs_out[hp][:])
    xr = [st.prw.tile([128, 512], f32, name="xr", tag=f"xr{i}") for i in range(4)]
    for tch in range(2):
        for cc in range(8):
            g = tch * 8 + cc
            if g % 4 == 0:
                for i in range(4):
                    ccx, tcx = (g + i) % 8, (g + i) // 8
                    nc.sync.dma_start(
                        xr[i][:], st.xrest_h[ccx * 128:(ccx + 1) * 128,
                                             tcx * 512:(tcx + 1) * 512])
            pg = st.ps_g()
            for k in range(4):
                nc.tensor.matmul(pg[:], st.wo_sb[k][:, cc * 128:(cc + 1) * 128],
                                 st.attA[k][:, tch * 512:(tch + 1) * 512],
                                 start=(k == 0), stop=False)
            for k in range(4):
                nc.tensor.matmul(pg[:], st.wo_sb[4 + k][:, cc * 128:(cc + 1) * 128],
                                 att_sb[k][:, tch * 512:(tch + 1) * 512],
                                 start=False, stop=(k == 3))
            nc.vector.tensor_tensor(st.x2T[g][:], pg[:], xr[g % 4][:], ALU.add)


def _emit_ln2_ffn(nc, st):
    """LN2 (column stats on x2T) + FFN with transposed output accumulated
    into x2T, then DMA out."""
    w = st.ffw
    h2T = [st.ffp.tile([128, 512], bf16, name="h2T", tag=f"h2T{i}")
           for i in range(16)]
    for tch in range(2):
        src = [st.x2T[tch * 8 + cc] for cc in range(8)]
        dst = [h2T[tch * 8 + cc] for cc in range(8)]
        _stats_and_norm(nc, st, src, 8, dst, "sq2", st.ffp, f32, st.onescolf,
                        sq_act=True)

    for g in range(4):
        w1g, w2g, utg = [], [], []
        for ff in range(8):
            f = g * 8 + ff
            w1c = w.tile([128, 1024], bf16, name="w1c", tag="w1c")
            nc.sync.dma_start(w1c[:], st.w1_h[f, :, :])
            w1g.append(w1c)
            w2t = w.tile([128, C], bf16, name="w2t", tag="w2t")
            nc.sync.dma_start(w2t[:], st.w2_h[f * 128:(f + 1) * 128, :])
            w2g.append(w2t)
        for ff in range(8):
            f = g * 8 + ff
            ut = w.tile([128, TH], bf16, name="ut", tag="ut")
            for tch in range(2):
                pg = st.ps_po(tch)
                for k in range(8):
                    nc.tensor.matmul(pg[:], w1g[ff][:, k * 128:(k + 1) * 128],
                                     h2T[tch * 8 + k][:],
                                     start=(k == 0), stop=(k == 7))
                nc.scalar.activation(ut[:, tch * 512:(tch + 1) * 512], pg[:],
                                     AF.Relu, bias=st.b1_sb[:, f:f + 1])
            utg.append(ut)
        for tch in range(2):
            for cc in range(8):
                pg = st.ps_g()
                for ff in range(8):
                    nc.tensor.matmul(pg[:], w2g[ff][:, cc * 128:(cc + 1) * 128],
                                     utg[ff][:, tch * 512:(tch + 1) * 512],
                                     start=(ff == 0),
                                     stop=(ff == 7 and g > 0))
                if g == 0:
                    nc.tensor.matmul(pg[:], st.b2_sb[:, cc * 128:(cc + 1) * 128],
                                     st.onesr[:, 0:512], start=False, stop=True)
                gidx = tch * 8 + cc
                nc.vector.tensor_tensor(st.x2T[gidx][:], pg[:],
                                        st.x2T[gidx][:], ALU.add)
                if g == 3:
                    nc.sync.dma_start(
                        st.y_h[cc * 128:(cc + 1) * 128,
                               tch * 512:(tch + 1) * 512],
                        st.x2T[gidx][:])


def build_program():
    if "nc" in _CACHE:
        return _CACHE["nc"]
    nc = bacc.Bacc(None)
    st = S()

    st.xt_h = nc.declare_dram_parameter("xt", [C, T], bf16, isOutput=False)
    st.xrest_h = nc.declare_dram_parameter("xrest", [C, TH], f32, isOutput=False)
    st.wq_h = nc.declare_dram_parameter("wq", [C, H], bf16, isOutput=False)
    st.wk_h = nc.declare_dram_parameter("wk", [C, H], bf16, isOutput=False)
    st.wv_h = nc.declare_dram_parameter("wv", [C, H], bf16, isOutput=False)
    bq_h = nc.declare_dram_parameter("bq", [128, 4], f32, isOutput=False)
    bk_h = nc.declare_dram_parameter("bk", [128, 4], f32, isOutput=False)
    bv_h = nc.declare_dram_parameter("bv", [1, H], bf16, isOutput=False)
    st.wo_h = nc.declare_dram_parameter("wo", [C, C], bf16, isOutput=False)
    st.w1_h = nc.declare_dram_parameter("w1", [32, 128, 1024], bf16,
                                        isOutput=False)
    b1_h = nc.declare_dram_parameter("b1", [128, 32], f32, isOutput=False)
    st.w2_h = nc.declare_dram_parameter("w2", [F, C], bf16, isOutput=False)
    b2_h = nc.declare_dram_parameter("b2", [1, C], bf16, isOutput=False)
    tri_h = nc.declare_dram_parameter("tri", [128, 128], bf16, isOutput=False)
    onesr_h = nc.declare_dram_parameter("onesr", [1, 512], bf16, isOutput=False)
    onescol_h = nc.declare_dram_parameter("onescol", [128, 1], bf16,
                                          isOutput=False)
    onescolf_h = nc.declare_dram_parameter("onescolf", [128, 1], f32,
                                           isOutput=False)
    ones8_h = nc.declare_dram_parameter("ones8", [128, 8], bf16, isOutput=False)
    sel_h = nc.declare_dram_parameter("sel", [128, 2], f32, isOutput=False)
    seln_h = nc.declare_dram_parameter("seln", [128, 2], f32, isOutput=False)
    st.y_h = nc.declare_dram_parameter("y", [C, TH], f32, isOutput=True)

    st.rs_in = [nc.dram_tensor(f"rs_in{hp}", [2, 128, TH], bf16)
                for hp in range(4)]
    st.rs_out = [nc.dram_tensor(f"rs_out{hp}", [128, TH], bf16)
                 for hp in range(4)]

    with tile.TileContext(nc) as tc, ExitStack() as stack:
        st.tc = tc
        cst = stack.enter_context(tc.tile_pool(name="const", bufs=1))
        # PSUM pools: psA single-buffered (pqk 2 banks + poA + poB),
        # psB double-buffered (gemm + aux) -> 4 + 4 = 8 banks.
        psA = stack.enter_context(tc.tile_pool(name="psA", bufs=1, space="PSUM"))
        psB = stack.enter_context(tc.tile_pool(name="psB", bufs=2, space="PSUM"))
        st.work = stack.enter_context(tc.tile_pool(name="work", bufs=2))
        st.rows = stack.enter_context(tc.tile_pool(name="rows", bufs=1))

        def ps_g(shape=(128, 512)):
            return psB.tile(list(shape), f32, tag="gemm", name="ps_gemm")

        def ps_aux(shape):
            return psB.tile(list(shape), f32, tag="aux", name="ps_aux")

        def ps_qk():
            return psA.tile([128, 1024], f32, tag="pqk", name="ps_qk")

        def ps_po(i):
            return psA.tile([128, 512], f32, tag=f"po{i}", name=f"ps_po{i}")
        st.ps_g, st.ps_aux, st.ps_qk, st.ps_po = ps_g, ps_aux, ps_qk, ps_po

        st.tri = cst.tile([128, 128], bf16, name="tri")
        st.onesr = cst.tile([1, 512], bf16, name="onesr")
        st.onescol = cst.tile([128, 1], bf16, name="onescol")
        st.onescolf = cst.tile([128, 1], f32, name="onescolf")
        st.ones8 = cst.tile([128, 8], bf16, name="ones8")
        st.bq_sb = cst.tile([128, 4], f32, name="bq_sb")
        st.bk_sb = cst.tile([128, 4], f32, name="bk_sb")
        st.bv_sb = cst.tile([1, H], bf16, name="bv_sb")
        st.b1_sb = cst.tile([128, 32], f32, name="b1_sb")
        st.b2_sb = cst.tile([1, C], bf16, name="b2_sb")
        st.sel_sb = cst.tile([128, 2], f32, name="sel_sb")
        st.seln_sb = cst.tile([128, 2], f32, name="seln_sb")
        for t_, h_ in [(st.tri, tri_h), (st.onesr, onesr_h),
                       (st.onescol, onescol_h), (st.onescolf, onescolf_h),
                       (st.ones8, ones8_h),
                       (st.bq_sb, bq_h), (st.bk_sb, bk_h), (st.bv_sb, bv_h),
                       (st.b1_sb, b1_h), (st.b2_sb, b2_h),
                       (st.sel_sb, sel_h), (st.seln_sb, seln_h)]:
            nc.sync.dma_start(t_[:], h_[:])

        resid = stack.enter_context(tc.tile_pool(name="resid", bufs=1))
        st.x2T = [resid.tile([128, 512], f32, name=f"x2T{i}", tag=f"x2T{i}")
                  for i in range(16)]

        with tc.tile_pool(name="persist", bufs=1) as persist:
            st.qT = [persist.tile([128, T], bf16, name=f"qT{i}", tag=f"qT{i}")
                     for i in range(4)]
            st.kT = [persist.tile([128, T], bf16, name=f"kT{i}", tag=f"kT{i}")
                     for i in range(4)]
            st.vn = [persist.tile([128, 520], bf16, name=f"vn{i}", tag=f"vn{i}")
                     for i in range(16)]
            st.attA = [persist.tile([128, TH], bf16, name=f"attA{i}",
                                    tag=f"attA{i}") for i in range(4)]
            st.wo_sb = [persist.tile([128, C], bf16, name=f"wo{k}", tag=f"wo{k}")
                        for k in range(8)]

            with tc.tile_pool(name="qkvw", bufs=1) as qkvw, \
                 tc.tile_pool(name="xh", bufs=2) as xh, \
                 tc.tile_pool(name="ptp", bufs=3) as ptp, \
                 tc.tile_pool(name="aw", bufs=2) as aw:
                st.wq_sb = [qkvw.tile([128, H], bf16, name=f"wq{k}",
                                      tag=f"wq{k}") for k in range(8)]
                st.wk_sb = [qkvw.tile([128, H], bf16, name=f"wk{k}",
                                      tag=f"wk{k}") for k in range(8)]
                st.wv_sb = [qkvw.tile([128, H], bf16, name=f"wv{k}",
                                      tag=f"wv{k}") for k in range(8)]
                st.xh, st.ptp, st.aw = xh, ptp, aw
                # chunk-0 x tiles first: the stat matmuls are the kernel's
                # first PE work, so their DMA must lead the Sync queue
                xt0 = _dma_xt_chunk(nc, st, 0)
                for k in range(8):
                    nc.sync.dma_start(st.wq_sb[k][:],
                                      st.wq_h[k * 128:(k + 1) * 128, :])
                    nc.sync.dma_start(st.wk_sb[k][:],
                                      st.wk_h[k * 128:(k + 1) * 128, :])
                    nc.sync.dma_start(st.wv_sb[k][:],
                                      st.wv_h[k * 128:(k + 1) * 128, :])
                for tt in range(16):
                    nc.sync.dma_start(
                        st.vn[tt][:, 0:520].rearrange("p (h e) -> p h e",
                                                      h=8)[:, :, 64:65],
                        st.ones8[:].rearrange("p (h o) -> p h o", h=8))

                for jc in range(4):
                    _emit_qkv_chunk(nc, st, jc, xt_pre=(xt0 if jc == 0 else None))
                    _emit_attn_chunk(nc, st, jc)
                    if jc == 1:
                        for k in range(8):
                            nc.sync.dma_start(st.wo_sb[k][:],
                                              st.wo_h[k * 128:(k + 1) * 128, :])

            with tc.tile_pool(name="prw", bufs=1) as prw:
                st.prw = prw
                _emit_proj(nc, st)

        with tc.tile_pool(name="ffp", bufs=1) as ffp, \
             tc.tile_pool(name="ffw", bufs=18) as ffw:
            st.ffp, st.ffw = ffp, ffw
            _emit_ln2_ffn(nc, st)

    nc.compile()
    _CACHE["nc"] = nc
    return nc


def make_inputs(x, Wq, Wk, Wv, Wo, bo, W1, b1, W2, b2,
                ln1_g, ln1_b, ln2_g, ln2_b):
    """Build per-core input maps (host-side sharding, transposes, LN folding)."""
    x = np.asarray(x, np.float32)
    scale = float(C) ** -0.5

    wq_eff = ln1_g[:, None] * Wq
    wk_eff = ln1_g[:, None] * Wk * scale
    wv_eff = ln1_g[:, None] * Wv
    bq_full = ln1_b @ Wq
    bk_full = (ln1_b @ Wk) * scale
    bv_full = ln1_b @ Wv
    w1_eff = ln2_g[:, None] * W1
    b1_eff = b1 + ln2_b @ W1

    BF = ml_dtypes.bfloat16
    tri = np.triu(np.ones((128, 128), BF))
    onesr = np.ones((1, 512), BF)
    onescol = np.ones((128, 1), BF)
    ones8 = np.ones((128, 8), BF)
    # W1 pre-shuffled: w1s[f, c, k*128+j] = w1_eff[k*128+c, f*128+j]
    w1s = np.ascontiguousarray(
        w1_eff.reshape(8, 128, 32, 128).transpose(2, 1, 0, 3)
        .reshape(32, 128, 1024).astype(BF))
    w2c = np.ascontiguousarray(W2.astype(BF))
    b1c = np.ascontiguousarray(b1_eff.reshape(32, 128).T)
    b2c = np.ascontiguousarray(b2.reshape(1, C).astype(BF))

    in_maps = []
    for core in range(8):
        b, s = core // 2, core % 2
        cs = slice(s * H, (s + 1) * H)
        ts = slice(s * TH, (s + 1) * TH)
        own = np.arange(s * H, (s + 1) * H)
        other = np.arange((1 - s) * H, (2 - s) * H)
        perm = np.concatenate([own, other])
        in_maps.append({
            "xt": np.ascontiguousarray(x[b].T.astype(BF)),
            "xrest": np.ascontiguousarray((x[b, ts, :] + bo[None, :]).T),
            "wq": np.ascontiguousarray(wq_eff[:, cs].astype(BF)),
            "wk": np.ascontiguousarray(wk_eff[:, cs].astype(BF)),
            "wv": np.ascontiguousarray(wv_eff[:, cs].astype(BF)),
            "bq": np.ascontiguousarray(bq_full[cs].reshape(4, 128).T),
            "bk": np.ascontiguousarray(bk_full[cs].reshape(4, 128).T),
            "bv": np.ascontiguousarray(bv_full[cs].reshape(1, H).astype(BF)),
            "wo": np.ascontiguousarray(Wo[perm, :].astype(BF)),
            "w1": w1s, "b1": b1c, "w2": w2c, "b2": b2c,
            "tri": tri, "onesr": onesr, "onescol": onescol,
            "onescolf": np.ones((128, 1), np.float32), "ones8": ones8,
            "sel": np.tile(np.eye(2, dtype=np.float32)[s][None, :], (128, 1)),
            "seln": np.tile(np.eye(2, dtype=np.float32)[1 - s][None, :], (128, 1)),
        })
    return in_maps


def kernel(**inputs):
    nc = build_program()
    in_maps = make_inputs(**{k: np.asarray(v, np.float32)
                             for k, v in inputs.items()})
    res = run_bass_kernel_spmd(nc, in_maps, list(range(8)))
    out = np.empty((B, T, C), np.float32)
    for core in range(8):
        b, s = core // 2, core % 2
        out[b, s * TH:(s + 1) * TH, :] = res.results[core]["y"].T
    return out


# revision 52
# speedup vs baseline: 1.4748x; 1.0800x over previous
"""Trainium2 Bass kernel for a dense transformer block (B=4, T=2048, C=1024, 16 heads).

Sharding over 8 NeuronCores: core i handles batch b=i//2 with shard s=i%2.
 - LN1 + QKV + causal attention for its 8 heads (c-slice [512s, 512s+512)) over full T
 - exchange of attention outputs within the (b) pair via 4 per-head-pair
   ReduceScatter ops (zero-masked, SPMD-symmetric)
 - proj + LN2 + FFN + residuals on its t-half rows [1024s, 1024s+1024)

All activations flow in TRANSPOSED layout [C, T] (host passes xT / xresT and
un-transposes y), which removes every PE transpose.  LayerNorm is computed as
column statistics (partition-sum matmuls) + broadcast normalize on DVE.
LayerNorm gains/biases are folded into the weights on the host.
"""

from contextlib import ExitStack

import ml_dtypes
import numpy as np

import concourse.bass as bass
import concourse.mybir as mybir
import concourse.tile as tile
from concourse import bacc
from concourse.bass_utils import run_bass_kernel_spmd

f32 = mybir.dt.float32
bf16 = mybir.dt.bfloat16
AF = mybir.ActivationFunctionType
ALU = mybir.AluOpType
AX = mybir.AxisListType

B, T, C = 4, 2048, 1024
NH, D = 16, 64
F = 4 * C
H = C // 2            # per-core head c-slice (8 heads)
TH = T // 2           # per-core t-half for proj/FFN
NC_CHUNK = 512        # t-chunk for qkv/attention
EPS = 1e-5
RG = [[0, 1], [2, 3], [4, 5], [6, 7]]

_CACHE = {}


class S:
    pass


def _stats_and_norm(nc, st, src_tiles, n_src, dst_tiles, sq_tag, pool, sq_dt,
                    ones_lhs, sq_act=False):
    """Column LayerNorm: src [128, 512] tiles covering 1024 c-rows -> dst bf16
    tiles (x - mu) * rsig per column.  sq_dt/ones_lhs match the src dtype so
    matmul operand dtypes agree.  sq_act routes squares to ACT (use only when
    ACT is not busy with attention exp)."""
    work = st.work
    sqs = []
    for k in range(n_src):
        sq = pool.tile([128, 512], sq_dt, name="sq", tag=f"{sq_tag}{k % 2}")
        if sq_act:
            nc.scalar.activation(sq[:], src_tiles[k][:], AF.Square)
        else:
            nc.vector.tensor_tensor(sq[:], src_tiles[k][:], src_tiles[k][:],
                                    ALU.mult)
        sqs.append(sq)
    # partition sums via M=1 matmuls into one bank: sum at partition 0, sumsq
    # at partition 32.  A single whole-tile copy is the only PSUM reader, so
    # no engine reads the bank while the PE is still writing it.
    pst = st.ps_aux((33, 512))
    for k in range(n_src):
        nc.tensor.matmul(pst[0:1, :], ones_lhs[:], src_tiles[k][:],
                         start=(k == 0), stop=False)
        nc.tensor.matmul(pst[32:33, :], ones_lhs[:], sqs[k][:],
                         start=False, stop=(k == n_src - 1))
    stat_sb = work.tile([33, 512], f32, name="stat_sb", tag="stat_sb")
    nc.vector.tensor_copy(out=stat_sb[:], in_=pst[:])
    # row math (mu/sd rows in bf16 so the broadcast matmuls match onesr's dtype)
    mu = st.rows.tile([1, 512], bf16, name="mu_r", tag="mu_r")
    msq = st.rows.tile([1, 512], f32, name="msq_r", tag="msq_r")
    var = st.rows.tile([1, 512], f32, name="var_r", tag="var_r")
    sd = st.rows.tile([1, 512], bf16, name="sd_r", tag="sd_r")
    nc.vector.tensor_scalar_mul(mu[:], stat_sb[0:1, :], 1.0 / C)
    nc.vector.tensor_scalar_mul(msq[:], stat_sb[32:33, :], 1.0 / C)
    nc.vector.tensor_tensor(var[:], mu[:], mu[:], ALU.mult)
    nc.vector.tensor_tensor(var[:], msq[:], var[:], ALU.subtract)
    nc.vector.tensor_scalar_add(var[:], var[:], EPS)
    nc.scalar.activation(sd[:], var[:], AF.Sqrt)
    # broadcasts
    pmu = st.ps_aux((128, 512))
    nc.tensor.matmul(pmu[:], st.onesr[:, 0:128], mu[:], start=True, stop=True)
    mu_b = work.tile([128, 512], bf16, name="mu_b", tag="mu_b")
    nc.vector.tensor_copy(out=mu_b[:], in_=pmu[:])
    psd = st.ps_aux((128, 512))
    nc.tensor.matmul(psd[:], st.onesr[:, 0:128], sd[:], start=True, stop=True)
    rsig_b = work.tile([128, 512], f32, name="rsig_b", tag="rsig_b")
    nc.vector.reciprocal_approx_fast(rsig_b[:], psd[:])
    rsig_bb = work.tile([128, 512], bf16, name="rsig_bb", tag="rsig_bb")
    nc.vector.tensor_copy(out=rsig_bb[:], in_=rsig_b[:])
    # normalize (all-bf16 tensor_tensor gets the DVE 2x mode, no implicit casts)
    for k in range(n_src):
        dif = pool.tile([128, 512], bf16, name="dif", tag=f"{sq_tag}{k % 2}")
        nc.vector.tensor_tensor(dif[:], src_tiles[k][:], mu_b[:], ALU.subtract)
        nc.vector.tensor_tensor(dst_tiles[k][:], dif[:], rsig_bb[:], ALU.mult)


def _dma_xt_chunk(nc, st, jc):
    xt = []
    for k in range(8):
        t_ = st.xh.tile([128, 512], bf16, name="xt", tag=f"xt{k}")
        nc.sync.dma_start(t_[:], st.xt_h[k * 128:(k + 1) * 128,
                                          jc * 512:(jc + 1) * 512])
        xt.append(t_)
    return xt


def _emit_qkv_chunk(nc, st, jc, xt_pre=None):
    """LN1 stats + q/k/v GEMMs for t-chunk jc (columns [jc*512, jc*512+512))."""
    xh, ps_g = st.xh, st.ps_g
    xt = xt_pre if xt_pre is not None else _dma_xt_chunk(nc, st, jc)
    hT = [xh.tile([128, 512], bf16, name="hT", tag=f"hT{k}") for k in range(8)]
    _stats_and_norm(nc, st, xt, 8, hT, "sq", xh, bf16, st.onescol)

    # q, k GEMMs -> transposed layout [c' 128, t 512]
    for dst, wsb, bsb in ((st.qT, st.wq_sb, st.bq_sb), (st.kT, st.wk_sb, st.bk_sb)):
        for co in range(4):
            pg = ps_g()
            for k in range(8):
                nc.tensor.matmul(pg[:], wsb[k][:, co * 128:(co + 1) * 128],
                                 hT[k][:], start=(k == 0), stop=(k == 7))
            nc.vector.tensor_scalar_add(dst[co][:, jc * 512:(jc + 1) * 512],
                                        pg[:], bsb[:, co:co + 1])
    # v GEMM -> natural layout [t 128, cv 512], strided into vn (65-col heads)
    for tt4 in range(4):
        tt = jc * 4 + tt4
        pg = ps_g()
        for k in range(8):
            nc.tensor.matmul(pg[:], hT[k][:, tt4 * 128:(tt4 + 1) * 128],
                             st.wv_sb[k][:], start=(k == 0), stop=False)
        nc.tensor.matmul(pg[:], st.onesr[:, 0:128], st.bv_sb[:],
                         start=False, stop=True)
        nc.vector.tensor_copy(
            out=st.vn[tt][:, 0:520].rearrange("p (h e) -> p h e", h=8)[:, :, 0:64],
            in_=pg[:].rearrange("p (h d) -> p h d", h=8))


def _emit_attn_chunk(nc, st, j):
    """Causal attention of q-chunk j against k-chunks 0..j, all 4 head-pairs.
    Writes the own-t-half part into compact attA and DMAs the peer part into
    rs_in; fires the per-hp ReduceScatter after the last chunk."""
    aw = st.aw
    tq0 = j * 512
    nk = 4 * (j + 1)
    sj = j // 2        # which t-half this q-chunk belongs to
    lc = (j % 2) * 512  # column inside the compact [128, TH] buffers
    for hp in range(4):
        po = [st.ps_po(0), st.ps_po(1)]
        for kk in range(nk):
            r = 128 * (kk - 4 * j) if kk >= 4 * j else 0
            pqk = st.ps_qk()
            for bi, b0 in enumerate((0, 64)):
                nc.tensor.matmul(
                    pqk[:, bi * 512 + r:bi * 512 + 512],
                    st.kT[hp][b0:b0 + 64, kk * 128:(kk + 1) * 128],
                    st.qT[hp][b0:b0 + 64, tq0 + r:tq0 + 512],
                    start=True, stop=True)
            ptb = st.ptp.tile([128, 1024], bf16, name="ptb", tag="pt")
            if r == 0:
                nc.scalar.activation(ptb[:], pqk[:], AF.Exp)
            else:
                nc.scalar.activation(
                    ptb[:].rearrange("p (b w) -> p b w", b=2)[:, :, r:512],
                    pqk[:].rearrange("p (b w) -> p b w", b=2)[:, :, r:512],
                    AF.Exp)
            if kk >= 4 * j:
                nc.vector.tensor_tensor(
                    ptb[:].rearrange("p (b w) -> p b w", b=2)[:, :, r:r + 128],
                    ptb[:].rearrange("p (b w) -> p b w", b=2)[:, :, r:r + 128],
                    st.tri[:, None, :].to_broadcast((128, 2, 128)),
                    ALU.mult)
            for bi in range(2):
                h = 2 * hp + bi
                nc.tensor.matmul(
                    po[bi][0:65, r:512],
                    st.vn[kk][:, 65 * h:65 * h + 65],
                    ptb[:, bi * 512 + r:bi * 512 + 512],
                    start=(kk == 0), stop=(kk == nk - 1))
        for bi, b0 in enumerate((0, 64)):
            dr = aw.tile([1, 512], bf16, name="dr", tag="dr")
            nc.scalar.copy(dr[:], po[bi][64:65, :])
            pb = st.ps_pb()
            nc.tensor.matmul(pb[:], st.onesr[:, 0:64], dr[:],
                             start=True, stop=True)
            rbi = aw.tile([64, 512], f32, name="rbi", tag="rbi")
            nc.vector.reciprocal_approx_fast(rbi[:], pb[:])
            tmp = aw.tile([64, 512], bf16, name="tmp", tag="tmp")
            nc.vector.tensor_tensor(tmp[:], po[bi][0:64, :], rbi[:], ALU.mult)
            asl = st.attA[hp][b0:b0 + 64, lc:lc + 512]
            if j < 2:
                nc.vector.tensor_scalar_mul(asl, tmp[:],
                                            st.sel_sb[0:64, sj:sj + 1])
            else:
                # stage at the same base partition as asl (tensor_tensor
                # requires equal base partitions for two SBUF inputs)
                tm2 = aw.tile([128, 512], bf16, name="tm2", tag="tm2")
                nc.vector.tensor_scalar_mul(tm2[b0:b0 + 64, :], tmp[:],
                                            st.sel_sb[0:64, sj:sj + 1])
                nc.vector.tensor_tensor(asl, asl, tm2[b0:b0 + 64, :], ALU.add)
            abc = aw.tile([64, 512], bf16, name="abc", tag="abc")
            nc.vector.tensor_scalar_mul(abc[:], tmp[:],
                                        st.seln_sb[0:64, sj:sj + 1])
            nc.sync.dma_start(
                st.rs_in[hp][sj, b0:b0 + 64, lc:lc + 512], abc[:])
        if j == 3:
            nc.gpsimd.collective_compute(
                "ReduceScatter", ALU.add, replica_groups=RG,
                ins=[st.rs_in[hp][:]], outs=[st.rs_out[hp][:]])


def _emit_proj(nc, st):
    """x2T = xresT + (att @ Wo)^T, transposed output [C, TH] in f32 (doubles
    as the FFN residual accumulator)."""
    att_sb = [st.prw.tile([128, TH], bf16, name=f"asb{k}", tag=f"asb{k}")
              for k in range(4)]
    for hp in range(4):
        nc.sync.dma_start(att_sb[hp][:], st.rs_out[hp][:])
    xr = [st.prw.tile([128, 512], f32, name="xr", tag=f"xr{i}") for i in range(4)]
    for tch in range(2):
        for cc in range(8):
            g = tch * 8 + cc
            if g % 4 == 0:
                for i in range(4):
                    ccx, tcx = (g + i) % 8, (g + i) // 8
                    nc.sync.dma_start(
                        xr[i][:], st.xrest_h[ccx * 128:(ccx + 1) * 128,
                                             tcx * 512:(tcx + 1) * 512])
            pg = st.ps_rr(g)
            for k in range(4):
                nc.tensor.matmul(pg[:], st.wo_sb[k][:, cc * 128:(cc + 1) * 128],
                                 st.attA[k][:, tch * 512:(tch + 1) * 512],
                                 start=(k == 0), stop=False)
            for k in range(4):
                nc.tensor.matmul(pg[:], st.wo_sb[4 + k][:, cc * 128:(cc + 1) * 128],
                                 att_sb[k][:, tch * 512:(tch + 1) * 512],
                                 start=False, stop=(k == 3))
            nc.vector.tensor_tensor(st.x2T[g][:], pg[:], xr[g % 4][:], ALU.add)


def _emit_ln2_ffn(nc, st):
    """LN2 (column stats on x2T) + FFN with transposed output accumulated
    into x2T, then DMA out."""
    w = st.ffw
    h2T = [st.ffp.tile([128, 512], bf16, name="h2T", tag=f"h2T{i}")
           for i in range(16)]
    for tch in range(2):
        src = [st.x2T[tch * 8 + cc] for cc in range(8)]
        dst = [h2T[tch * 8 + cc] for cc in range(8)]
        _stats_and_norm(nc, st, src, 8, dst, "sq2", st.ffp, f32, st.onescolf,
                        sq_act=True)

    for g in range(4):
        w1g, w2g, utg = [], [], []
        for ff in range(8):
            f = g * 8 + ff
            w1c = w.tile([128, 1024], bf16, name="w1c", tag="w1c")
            nc.sync.dma_start(w1c[:], st.w1_h[f, :, :])
            w1g.append(w1c)
            w2t = w.tile([128, C], bf16, name="w2t", tag="w2t")
            nc.sync.dma_start(w2t[:], st.w2_h[f * 128:(f + 1) * 128, :])
            w2g.append(w2t)
        for ff in range(8):
            f = g * 8 + ff
            ut = w.tile([128, TH], bf16, name="ut", tag="ut")
            for tch in range(2):
                pg = st.ps_rr(ff * 2 + tch)
                for k in range(8):
                    nc.tensor.matmul(pg[:], w1g[ff][:, k * 128:(k + 1) * 128],
                                     h2T[tch * 8 + k][:],
                                     start=(k == 0), stop=(k == 7))
                nc.scalar.activation(ut[:, tch * 512:(tch + 1) * 512], pg[:],
                                     AF.Relu, bias=st.b1_sb[:, f:f + 1])
            utg.append(ut)
        for tch in range(2):
            for cc in range(8):
                pg = st.ps_rr(tch * 8 + cc + 3)
                for ff in range(8):
                    nc.tensor.matmul(pg[:], w2g[ff][:, cc * 128:(cc + 1) * 128],
                                     utg[ff][:, tch * 512:(tch + 1) * 512],
                                     start=(ff == 0),
                                     stop=(ff == 7 and g > 0))
                if g == 0:
                    nc.tensor.matmul(pg[:], st.b2_sb[:, cc * 128:(cc + 1) * 128],
                                     st.onesr[:, 0:512], start=False, stop=True)
                gidx = tch * 8 + cc
                nc.vector.tensor_tensor(st.x2T[gidx][:], pg[:],
                                        st.x2T[gidx][:], ALU.add)
                if g == 3:
                    nc.sync.dma_start(
                        st.y_h[cc * 128:(cc + 1) * 128,
                               tch * 512:(tch + 1) * 512],
                        st.x2T[gidx][:])


def build_program():
    if "nc" in _CACHE:
        return _CACHE["nc"]
    nc = bacc.Bacc(None)
    st = S()

    st.xt_h = nc.declare_dram_parameter("xt", [C, T], bf16, isOutput=False)
    st.xrest_h = nc.declare_dram_parameter("xrest", [C, TH], f32, isOutput=False)
    st.wq_h = nc.declare_dram_parameter("wq", [C, H], bf16, isOutput=False)
    st.wk_h = nc.declare_dram_parameter("wk", [C, H], bf16, isOutput=False)
    st.wv_h = nc.declare_dram_parameter("wv", [C, H], bf16, isOutput=False)
    bq_h = nc.declare_dram_parameter("bq", [128, 4], f32, isOutput=False)
    bk_h = nc.declare_dram_parameter("bk", [128, 4], f32, isOutput=False)
    bv_h = nc.declare_dram_parameter("bv", [1, H], bf16, isOutput=False)
    st.wo_h = nc.declare_dram_parameter("wo", [C, C], bf16, isOutput=False)
    st.w1_h = nc.declare_dram_parameter("w1", [32, 128, 1024], bf16,
                                        isOutput=False)
    b1_h = nc.declare_dram_parameter("b1", [128, 32], f32, isOutput=False)
    st.w2_h = nc.declare_dram_parameter("w2", [F, C], bf16, isOutput=False)
    b2_h = nc.declare_dram_parameter("b2", [1, C], bf16, isOutput=False)
    tri_h = nc.declare_dram_parameter("tri", [128, 128], bf16, isOutput=False)
    onesr_h = nc.declare_dram_parameter("onesr", [1, 512], bf16, isOutput=False)
    onescol_h = nc.declare_dram_parameter("onescol", [128, 1], bf16,
                                          isOutput=False)
    onescolf_h = nc.declare_dram_parameter("onescolf", [128, 1], f32,
                                           isOutput=False)
    ones8_h = nc.declare_dram_parameter("ones8", [128, 8], bf16, isOutput=False)
    sel_h = nc.declare_dram_parameter("sel", [128, 2], f32, isOutput=False)
    seln_h = nc.declare_dram_parameter("seln", [128, 2], f32, isOutput=False)
    st.y_h = nc.declare_dram_parameter("y", [C, TH], f32, isOutput=True)

    st.rs_in = [nc.dram_tensor(f"rs_in{hp}", [2, 128, TH], bf16)
                for hp in range(4)]
    st.rs_out = [nc.dram_tensor(f"rs_out{hp}", [128, TH], bf16)
                 for hp in range(4)]

    with tile.TileContext(nc) as tc, ExitStack() as stack:
        st.tc = tc
        cst = stack.enter_context(tc.tile_pool(name="const", bufs=1))
        # PSUM pools: psA single-buffered (pqk 2 banks + poA + poB),
        # psB double-buffered (gemm + aux) -> 4 + 4 = 8 banks.
        psA = stack.enter_context(tc.tile_pool(name="psA", bufs=1, space="PSUM"))
        psB = stack.enter_context(tc.tile_pool(name="psB", bufs=2, space="PSUM"))
        st.work = stack.enter_context(tc.tile_pool(name="work", bufs=2))
        st.rows = stack.enter_context(tc.tile_pool(name="rows", bufs=1))

        def ps_g(shape=(128, 512)):
            return psB.tile(list(shape), f32, tag="gemm", name="ps_gemm")

        def ps_aux(shape):
            return psA.tile(list(shape), f32, tag="aux", name="ps_aux")

        def ps_qk():
            return psA.tile([128, 1024], f32, tag="pqk", name="ps_qk")

        def ps_po(i):
            return psA.tile([128, 512], f32, tag=f"po{i}", name=f"ps_po{i}")

        def ps_pb():
            return psA.tile([64, 512], f32, tag="pb", name="ps_pb")

        def ps_rr(i, shape=(128, 512)):
            # round-robin across every idle bank for deep group pipelining
            # (used by proj / FFN, when the attention tags are free)
            tags = ["gemm", "gemm", "pqk", "po0", "po1", "pb"]
            pool = psB if tags[i % 6] == "gemm" else psA
            return pool.tile(list(shape), f32, tag=tags[i % 6], name="ps_rr")
        st.ps_g, st.ps_aux, st.ps_qk, st.ps_po = ps_g, ps_aux, ps_qk, ps_po
        st.ps_pb, st.ps_rr = ps_pb, ps_rr

        st.tri = cst.tile([128, 128], bf16, name="tri")
        st.onesr = cst.tile([1, 512], bf16, name="onesr")
        st.onescol = cst.tile([128, 1], bf16, name="onescol")
        st.onescolf = cst.tile([128, 1], f32, name="onescolf")
        st.ones8 = cst.tile([128, 8], bf16, name="ones8")
        st.bq_sb = cst.tile([128, 4], f32, name="bq_sb")
        st.bk_sb = cst.tile([128, 4], f32, name="bk_sb")
        st.bv_sb = cst.tile([1, H], bf16, name="bv_sb")
        st.b1_sb = cst.tile([128, 32], f32, name="b1_sb")
        st.b2_sb = cst.tile([1, C], bf16, name="b2_sb")
        st.sel_sb = cst.tile([128, 2], f32, name="sel_sb")
        st.seln_sb = cst.tile([128, 2], f32, name="seln_sb")
        for t_, h_ in [(st.tri, tri_h), (st.onesr, onesr_h),
                       (st.onescol, onescol_h), (st.onescolf, onescolf_h),
                       (st.ones8, ones8_h),
                       (st.bq_sb, bq_h), (st.bk_sb, bk_h), (st.bv_sb, bv_h),
                       (st.b1_sb, b1_h), (st.b2_sb, b2_h),
                       (st.sel_sb, sel_h), (st.seln_sb, seln_h)]:
            nc.sync.dma_start(t_[:], h_[:])

        resid = stack.enter_context(tc.tile_pool(name="resid", bufs=1))
        st.x2T = [resid.tile([128, 512], f32, name=f"x2T{i}", tag=f"x2T{i}")
                  for i in range(16)]

        with tc.tile_pool(name="persist", bufs=1) as persist:
            st.qT = [persist.tile([128, T], bf16, name=f"qT{i}", tag=f"qT{i}")
                     for i in range(4)]
            st.kT = [persist.tile([128, T], bf16, name=f"kT{i}", tag=f"kT{i}")
                     for i in range(4)]
            st.vn = [persist.tile([128, 520], bf16, name=f"vn{i}", tag=f"vn{i}")
                     for i in range(16)]
            st.attA = [persist.tile([128, TH], bf16, name=f"attA{i}",
                                    tag=f"attA{i}") for i in range(4)]
            st.wo_sb = [persist.tile([128, C], bf16, name=f"wo{k}", tag=f"wo{k}")
                        for k in range(8)]

            with tc.tile_pool(name="qkvw", bufs=1) as qkvw, \
                 tc.tile_pool(name="xh", bufs=2) as xh, \
                 tc.tile_pool(name="ptp", bufs=3) as ptp, \
                 tc.tile_pool(name="aw", bufs=2) as aw:
                st.wq_sb = [qkvw.tile([128, H], bf16, name=f"wq{k}",
                                      tag=f"wq{k}") for k in range(8)]
                st.wk_sb = [qkvw.tile([128, H], bf16, name=f"wk{k}",
                                      tag=f"wk{k}") for k in range(8)]
                st.wv_sb = [qkvw.tile([128, H], bf16, name=f"wv{k}",
                                      tag=f"wv{k}") for k in range(8)]
                st.xh, st.ptp, st.aw = xh, ptp, aw
                # chunk-0 x tiles first: the stat matmuls are the kernel's
                # first PE work, so their DMA must lead the Sync queue
                xt0 = _dma_xt_chunk(nc, st, 0)
                for k in range(8):
                    nc.sync.dma_start(st.wq_sb[k][:],
                                      st.wq_h[k * 128:(k + 1) * 128, :])
                    nc.sync.dma_start(st.wk_sb[k][:],
                                      st.wk_h[k * 128:(k + 1) * 128, :])
                    nc.sync.dma_start(st.wv_sb[k][:],
                                      st.wv_h[k * 128:(k + 1) * 128, :])
                for tt in range(16):
                    nc.sync.dma_start(
                        st.vn[tt][:, 0:520].rearrange("p (h e) -> p h e",
                                                      h=8)[:, :, 64:65],
                        st.ones8[:].rearrange("p (h o) -> p h o", h=8))

                for jc in range(4):
                    _emit_qkv_chunk(nc, st, jc, xt_pre=(xt0 if jc == 0 else None))
                    _emit_attn_chunk(nc, st, jc)
                    if jc == 1:
                        for k in range(8):
                            nc.sync.dma_start(st.wo_sb[k][:],
                                              st.wo_h[k * 128:(k + 1) * 128, :])

            with tc.tile_pool(name="prw", bufs=1) as prw:
                st.prw = prw
                _emit_proj(nc, st)

        with tc.tile_pool(name="ffp", bufs=1) as ffp, \
             tc.tile_pool(name="ffw", bufs=18) as ffw:
            st.ffp, st.ffw = ffp, ffw
            _emit_ln2_ffn(nc, st)

    nc.compile()
    _CACHE["nc"] = nc
    return nc


def make_inputs(x, Wq, Wk, Wv, Wo, bo, W1, b1, W2, b2,
                ln1_g, ln1_b, ln2_g, ln2_b):
    """Build per-core input maps (host-side sharding, transposes, LN folding)."""
    x = np.asarray(x, np.float32)
    scale = float(C) ** -0.5

    wq_eff = ln1_g[:, None] * Wq
    wk_eff = ln1_g[:, None] * Wk * scale
    wv_eff = ln1_g[:, None] * Wv
    bq_full = ln1_b @ Wq
    bk_full = (ln1_b @ Wk) * scale
    bv_full = ln1_b @ Wv
    w1_eff = ln2_g[:, None] * W1
    b1_eff = b1 + ln2_b @ W1

    BF = ml_dtypes.bfloat16
    tri = np.triu(np.ones((128, 128), BF))
    onesr = np.ones((1, 512), BF)
    onescol = np.ones((128, 1), BF)
    ones8 = np.ones((128, 8), BF)
    # W1 pre-shuffled: w1s[f, c, k*128+j] = w1_eff[k*128+c, f*128+j]
    w1s = np.ascontiguousarray(
        w1_eff.reshape(8, 128, 32, 128).transpose(2, 1, 0, 3)
        .reshape(32, 128, 1024).astype(BF))
    w2c = np.ascontiguousarray(W2.astype(BF))
    b1c = np.ascontiguousarray(b1_eff.reshape(32, 128).T)
    b2c = np.ascontiguousarray(b2.reshape(1, C).astype(BF))

    in_maps = []
    for core in range(8):
        b, s = core // 2, core % 2
        cs = slice(s * H, (s + 1) * H)
        ts = slice(s * TH, (s + 1) * TH)
        own = np.arange(s * H, (s + 1) * H)
        other = np.arange((1 - s) * H, (2 - s) * H)
        perm = np.concatenate([own, other])
        in_maps.append({
            "xt": np.ascontiguousarray(x[b].T.astype(BF)),
            "xrest": np.ascontiguousarray((x[b, ts, :] + bo[None, :]).T),
            "wq": np.ascontiguousarray(wq_eff[:, cs].astype(BF)),
            "wk": np.ascontiguousarray(wk_eff[:, cs].astype(BF)),
            "wv": np.ascontiguousarray(wv_eff[:, cs].astype(BF)),
            "bq": np.ascontiguousarray(bq_full[cs].reshape(4, 128).T),
            "bk": np.ascontiguousarray(bk_full[cs].reshape(4, 128).T),
            "bv": np.ascontiguousarray(bv_full[cs].reshape(1, H).astype(BF)),
            "wo": np.ascontiguousarray(Wo[perm, :].astype(BF)),
            "w1": w1s, "b1": b1c, "w2": w2c, "b2": b2c,
            "tri": tri, "onesr": onesr, "onescol": onescol,
            "onescolf": np.ones((128, 1), np.float32), "ones8": ones8,
            "sel": np.tile(np.eye(2, dtype=np.float32)[s][None, :], (128, 1)),
            "seln": np.tile(np.eye(2, dtype=np.float32)[1 - s][None, :], (128, 1)),
        })
    return in_maps


def kernel(**inputs):
    nc = build_program()
    in_maps = make_inputs(**{k: np.asarray(v, np.float32)
                             for k, v in inputs.items()})
    res = run_bass_kernel_spmd(nc, in_maps, list(range(8)))
    out = np.empty((B, T, C), np.float32)
    for core in range(8):
        b, s = core // 2, core % 2
        out[b, s * TH:(s + 1) * TH, :] = res.results[core]["y"].T
    return out
